# revision 1
# baseline (speedup 1.0000x reference)
"""GIN graph encoder (DispatchGraphEncoder) on 8 Trainium2 NeuronCores. v2.

Strategy (node-sharded SPMD, gather-ucode-roofline design):
- Nodes split across 8 cores (12500 each, BLK=14336 = 112 windows of 128),
  LPT-balanced by in-degree per (core, window) bin.
- Global feature table T (bf16 [114688, 256]) is double-buffered in shared
  DRAM; each layer's table is produced by FOUR chunked AllGathers (one per
  window-group of 28 windows), so collectives overlap the gather stream.
- Aggregation: per segment (= window-group, 28672 rows, int16-addressable),
  each core's incident edges are sorted by dst window and packed DENSELY
  into 128-slot tiles (tiles may straddle windows). Only per-(seg,window)
  cross-core max padding remains (~9%). dma_gather pulls source rows; a
  one-hot S column-selector is built ON-CHIP per piece (DVE is_equal
  against an iota row constant) and right-multiplies each tile on the PE,
  accumulating per-window sums in PSUM across that window's pieces.
- z = agg + (1+eps)h built feature-major via PE transpose + scaled-identity
  trick; 2-layer GIN MLP feature-major, relu+bias fused on ACT; output
  transposed back node-major into the next h buffer (double-buffered).
- MLP pairs are emitted interleaved with the LAST segment's gather calls;
  next-layer AllGather chunks are emitted as their window groups complete,
  so gpsimd (the gather engine, the bottleneck) never idles on collectives.
- Pooling via graph-membership matmul, small AllReduce, replicated head.
"""
import sys

import numpy as np
import ml_dtypes

sys.path.insert(0, "/opt/trn_rl_repo")

from concourse import bass, bacc, mybir, tile  # noqa: E402
from concourse.masks import make_identity  # noqa: E402

P = 128


def full_cfg():
    return dict(
        N=100000, E=800000, D=128, H=256, OUT=512, L=4, G=64, NCORES=8,
        RN=12500, BLK=14336, NGRP=4, CALL_TILES=8,
    )


def tiny_cfg():
    return dict(
        N=2000, E=8192, D=128, H=256, OUT=512, L=2, G=8, NCORES=8,
        RN=250, BLK=512, NGRP=2, CALL_TILES=4,
    )


def derive(cfg):
    cfg = dict(cfg)
    cfg["NW"] = cfg["BLK"] // P
    assert cfg["NW"] % cfg["NGRP"] == 0
    cfg["WG"] = cfg["NW"] // cfg["NGRP"]          # windows per group
    cfg["GROWS"] = cfg["WG"] * P                  # h rows per group per core
    cfg["SEGLEN"] = cfg["NCORES"] * cfg["GROWS"]  # table rows per segment
    assert cfg["SEGLEN"] <= 32767
    cfg["TROWS"] = cfg["NGRP"] * cfg["SEGLEN"]
    nw = cfg["NW"]
    base, extra = cfg["RN"] // nw, cfg["RN"] % nw
    cfg["SCHED"] = [base + 1 if w < extra else base for w in range(nw)]
    assert max(cfg["SCHED"]) <= P
    return cfg


# --------------------------------------------------------------------------
# host-side preprocessing (pure index/metadata manipulation)
# --------------------------------------------------------------------------

def assign_nodes(cfg, indeg):
    """Degree-balanced LPT: node -> (core, window-slot position)."""
    import heapq
    c = cfg
    nw, ncores = c["NW"], c["NCORES"]
    sched = c["SCHED"]
    heap = []
    for core in range(ncores):
        for w in range(nw):
            heap.append((0.0, core * nw + w))
    heapq.heapify(heap)
    fill = np.zeros(ncores * nw, np.int64)
    n = len(indeg)
    node2core = np.empty(n, np.int64)
    node2pos = np.empty(n, np.int64)
    order = np.argsort(-indeg, kind="stable")
    for v in order:
        while True:
            load, b = heapq.heappop(heap)
            w = b % nw
            if fill[b] < sched[w]:
                break
        node2core[v] = b // nw
        node2pos[v] = w * P + fill[b]
        fill[b] += 1
        if fill[b] < sched[w]:
            heapq.heappush(heap, (load + float(indeg[v]), b))
    return node2core, node2pos


def preprocess(cfg, edge_index):
    """Build the uniform tile/piece program + per-core gather/S data.

    The layer loop runs window-BLOCK-major (one block = one AG window
    group), segments inner, so MLP/AG work spreads uniformly across the
    layer.  plan[b][s] = dict(tiles, calls, pieces, off, t_base, p_base);
    pieces are (w, t, first, last) in stream order with PSUM start/stop
    flags per (block, seg, window).
    """
    c = cfg
    src = np.asarray(edge_index[0], dtype=np.int64)
    dst = np.asarray(edge_index[1], dtype=np.int64)

    indeg = np.bincount(dst, minlength=c["N"])
    node2core, node2pos = assign_nodes(c, indeg)

    ncores, ngrp, nw, wg = c["NCORES"], c["NGRP"], c["NW"], c["WG"]
    seglen, grows = c["SEGLEN"], c["GROWS"]

    w_of = node2pos // P
    grp_of = w_of // wg
    rel_of = node2core * grows + (w_of % wg) * P + (node2pos % P)

    owner = node2core[dst]
    s_e = grp_of[src]
    rel_e = rel_of[src]
    dw_e = node2pos[dst] // P
    dc_e = node2pos[dst] % P

    counts = np.zeros((ncores, ngrp, nw), np.int64)
    np.add.at(counts, (owner, s_e, dw_e), 1)
    u = counts.max(axis=0)                        # [ngrp, nw] uniform counts

    order = np.lexsort((dw_e, s_e, owner))
    o_s, s_s, r_s, w_s, c_s = (owner[order], s_e[order], rel_e[order],
                               dw_e[order], dc_e[order])

    key = (o_s * ngrp + s_s) * nw + w_s
    bounds = np.flatnonzero(np.diff(key)) + 1
    starts = np.concatenate(([0], bounds))
    ends = np.concatenate((bounds, [len(key)]))
    range_of = {}
    for a, b in zip(starts, ends):
        range_of[(int(o_s[a]), int(s_s[a]), int(w_s[a]))] = (int(a), int(b))

    plan = [[None] * ngrp for _ in range(ngrp)]   # [block][seg]
    t_tot = 0
    p_tot = 0
    for blk in range(ngrp):
        wlo, whi = blk * wg, (blk + 1) * wg
        for s in range(ngrp):
            us = u[s, wlo:whi]
            off = np.concatenate(([0], np.cumsum(us)))
            length = int(off[-1])
            tiles = (length + P - 1) // P
            pieces = []
            for wi in range(wg):
                a, b = int(off[wi]), int(off[wi + 1])
                if a == b:
                    continue
                ta, tb = a // P, (b - 1) // P
                for t in range(ta, tb + 1):
                    pieces.append((wlo + wi, t, a >= t * P,
                                   b <= (t + 1) * P))
            calls = []
            t0 = 0
            while t0 < tiles:
                t1 = min(t0 + c["CALL_TILES"], tiles)
                calls.append((t0, t1))
                t0 = t1
            plan[blk][s] = dict(tiles=tiles, calls=calls, pieces=pieces,
                                off=off, t_base=t_tot, p_base=p_tot)
            t_tot += tiles
            p_tot += len(pieces)

    gidx = np.zeros((ncores, P, t_tot * 8), np.int16)
    s_u8 = np.zeros((ncores, p_tot, P, P), ml_dtypes.bfloat16)

    for blk in range(ngrp):
        wlo = blk * wg
        for s in range(ngrp):
            pl = plan[blk][s]
            off = pl["off"]
            t_base, p_base = pl["t_base"], pl["p_base"]
            for core in range(ncores):
                stream_rel = np.zeros(pl["tiles"] * P, np.int64)
                stream_col = np.full(pl["tiles"] * P, -1, np.int64)
                for wi in range(c["WG"]):
                    a = int(off[wi])
                    rng = range_of.get((core, s, wlo + wi))
                    if rng is None:
                        continue
                    ea, eb = rng
                    n = eb - ea
                    stream_rel[a:a + n] = r_s[ea:eb]
                    stream_col[a:a + n] = c_s[ea:eb]
                jj = np.arange(pl["tiles"] * P)
                tt = jj // P
                within = jj % P
                cols = (t_base + tt) * 8 + within // 16
                rows = within % 16
                for repl in range(8):
                    gidx[core, rows + 16 * repl, cols] = (
                        stream_rel.astype(np.int16))
                for i, (w, t, first, last) in enumerate(pl["pieces"]):
                    wi = w - wlo
                    a = max(int(off[wi]), t * P)
                    b = min(int(off[wi + 1]), (t + 1) * P)
                    lo = a - t * P
                    hi = b - t * P
                    sl = stream_col[t * P + lo:t * P + hi]
                    rws = np.arange(lo, hi)
                    m = sl >= 0
                    s_u8[core, p_base + i, rws[m], sl[m]] = 1

    return plan, gidx, s_u8, node2core, node2pos


def build_host_inputs(cfg, inputs):
    c = cfg
    x = np.asarray(inputs["x"], np.float32)
    batch = np.asarray(inputs["batch"], np.int64)
    plan, gidx, s_u8, node2core, node2pos = preprocess(c, inputs["edge_index"])

    L, H, D, OUT, G = c["L"], c["H"], c["D"], c["OUT"], c["G"]
    node_w = np.asarray(inputs["node_w"], np.float32)
    node_b = np.asarray(inputs["node_b"], np.float32)
    gw1 = np.asarray(inputs["gin_w1"], np.float32)
    gb1 = np.asarray(inputs["gin_b1"], np.float32)
    gw2 = np.asarray(inputs["gin_w2"], np.float32)
    gb2 = np.asarray(inputs["gin_b2"], np.float32)
    eps = np.asarray(inputs["eps"], np.float32)
    ow1 = np.asarray(inputs["out_w1"], np.float32)
    ob1 = np.asarray(inputs["out_b1"], np.float32)
    ow2 = np.asarray(inputs["out_w2"], np.float32)
    ob2 = np.asarray(inputs["out_b2"], np.float32)

    cnt = np.bincount(batch, minlength=G).astype(np.float32)

    common = {
        "wpT": np.ascontiguousarray(node_w.T),              # [D, H]
        "bpT": np.ascontiguousarray(node_b.reshape(H // P, P).T),   # [P, H/P]
        "w1T": np.ascontiguousarray(
            np.transpose(gw1, (0, 2, 1))).astype(ml_dtypes.bfloat16),
        "b1T": np.ascontiguousarray(np.transpose(
            gb1.reshape(L, H // P, P), (0, 2, 1))),          # [L, P, H/P]
        "w2T": np.ascontiguousarray(
            np.transpose(gw2, (0, 2, 1))).astype(ml_dtypes.bfloat16),
        "b2T": np.ascontiguousarray(np.transpose(
            gb2.reshape(L, H // P, P), (0, 2, 1))),
        "eps_rep": np.tile(eps.reshape(1, L), (P, 1)).astype(np.float32),
        "wo1T": np.ascontiguousarray(ow1.T),                # [H, H]
        "bo1T": np.ascontiguousarray(ob1.reshape(H // P, P).T),
        "wo2T": np.ascontiguousarray(ow2.T),                # [H, OUT]
        "bo2T": np.ascontiguousarray(ob2.reshape(OUT // P, P).T),   # [P, OUT/P]
        "cnt_rep": np.tile(cnt.reshape(1, G), (P, 1)),
    }

    in_maps = []
    for core in range(c["NCORES"]):
        mine = np.flatnonzero(node2core == core)
        pos = node2pos[mine]
        xo = np.zeros((D, c["BLK"]), np.float32)
        xo[:, pos] = x[mine].T
        gT = np.zeros((c["BLK"], G), ml_dtypes.bfloat16)
        gT[pos, batch[mine]] = 1.0
        m = dict(common)
        m["x_own"] = xo
        m["gidx"] = gidx[core]
        m["s_u8"] = s_u8[core]
        m["gT"] = gT
        in_maps.append(m)
    return plan, in_maps


# --------------------------------------------------------------------------
# device program
# --------------------------------------------------------------------------

def build_program(cfg, plan):
    c = cfg
    L, H, D, OUT, G = c["L"], c["H"], c["D"], c["OUT"], c["G"]
    NW, BLK, NGRP, WG = c["NW"], c["BLK"], c["NGRP"], c["WG"]
    GROWS, SEGLEN, TROWS = c["GROWS"], c["SEGLEN"], c["TROWS"]
    NH = H // P
    NO = OUT // P
    f32 = mybir.dt.float32
    bf16 = mybir.dt.bfloat16
    t_tot = sum(pl["tiles"] for row in plan for pl in row)
    p_tot = sum(len(pl["pieces"]) for row in plan for pl in row)

    nc = bacc.Bacc("TRN2", target_bir_lowering=False, debug=False)

    x_own = nc.dram_tensor("x_own", [D, BLK], f32, kind="ExternalInput")
    gidx_d = nc.dram_tensor("gidx", [P, t_tot * 8], mybir.dt.int16,
                            kind="ExternalInput")
    s_d = nc.dram_tensor("s_u8", [p_tot, P, P], bf16, kind="ExternalInput")
    gT_d = nc.dram_tensor("gT", [BLK, G], bf16, kind="ExternalInput")
    wpT_d = nc.dram_tensor("wpT", [D, H], f32, kind="ExternalInput")
    bpT_d = nc.dram_tensor("bpT", [P, NH], f32, kind="ExternalInput")
    w1T_d = nc.dram_tensor("w1T", [L, H, H], bf16, kind="ExternalInput")
    b1T_d = nc.dram_tensor("b1T", [L, P, NH], f32, kind="ExternalInput")
    w2T_d = nc.dram_tensor("w2T", [L, H, H], bf16, kind="ExternalInput")
    b2T_d = nc.dram_tensor("b2T", [L, P, NH], f32, kind="ExternalInput")
    eps_d = nc.dram_tensor("eps_rep", [P, L], f32, kind="ExternalInput")
    wo1T_d = nc.dram_tensor("wo1T", [H, H], f32, kind="ExternalInput")
    bo1T_d = nc.dram_tensor("bo1T", [P, NH], f32, kind="ExternalInput")
    wo2T_d = nc.dram_tensor("wo2T", [H, OUT], f32, kind="ExternalInput")
    bo2T_d = nc.dram_tensor("bo2T", [P, NO], f32, kind="ExternalInput")
    cnt_d = nc.dram_tensor("cnt_rep", [P, G], f32, kind="ExternalInput")

    out_d = nc.dram_tensor("out", [G, OUT], f32, kind="ExternalOutput")

    h_ab = [nc.dram_tensor(f"h{i}", [BLK, H], bf16) for i in range(2)]
    T_ab = [nc.dram_tensor(f"T{i}", [TROWS, H], bf16, addr_space="Shared")
            for i in range(2)]
    pp_in = nc.dram_tensor("pp_in", [P, NH * G], f32)
    pp_out = nc.dram_tensor("pp_out", [P, NH * G], f32, addr_space="Shared")

    rg = [list(range(c["NCORES"]))]
    sched = c["SCHED"]

    with tile.TileContext(nc) as tc:
        with (
            tc.tile_pool(name="const", bufs=1) as cpool,
            tc.tile_pool(name="agg", bufs=1) as apool,
            tc.tile_pool(name="wt", bufs=2) as wpool,
            tc.tile_pool(name="sb", bufs=3) as sb,
            tc.tile_pool(name="idx", bufs=3) as idxp,
            tc.tile_pool(name="gb", bufs=6) as gbp,
            tc.tile_pool(name="ssb", bufs=6) as ssp,
            tc.tile_pool(name="ps", bufs=3, space="PSUM") as ps,
            tc.tile_pool(name="ps_t", bufs=1, space="PSUM") as ps_t,
            tc.tile_pool(name="ps_agg", bufs=2, space="PSUM") as ps_agg,
            tc.tile_pool(name="pool_ps", bufs=1, space="PSUM") as ppool,
        ):
            ident = cpool.tile([P, P], f32)
            make_identity(nc, ident[:])
            identb = cpool.tile([P, P], bf16)
            nc.vector.tensor_copy(identb[:], ident[:])
            eps_t = cpool.tile([P, L], f32)
            nc.sync.dma_start(out=eps_t[:], in_=eps_d[:])
            eps1p = cpool.tile([P, L], f32)
            nc.scalar.add(eps1p[:], eps_t[:], 1.0)

            # zero both h buffers once (pad slots inside every window)
            ZC = min(8, NW)
            zt = cpool.tile([P, ZC * H], bf16)
            nc.gpsimd.memset(zt[:], 0)
            assert BLK % (ZC * P) == 0
            for hb in h_ab:
                for zb in range(BLK // (ZC * P)):
                    nc.sync.dma_start(
                        out=hb[zb * ZC * P:(zb + 1) * ZC * P, :].rearrange(
                            "(a p) c -> p a c", p=P),
                        in_=zt[:].rearrange("p (a c) -> p a c", c=H))

            def rows_of(w):
                return sched[w]

            def write_node_major(w, hfm_parts, hb, fm_f32=True,
                                 pool_into=None):
                """hfm_parts: list of NH [P, P] APs (feature-major) ->
                transpose -> node-major window w of h buffer hb."""
                if fm_f32:
                    htps = ps.tile([P, H], f32, space="PSUM", tag="mlp")
                    rid = ident[:]
                else:
                    htps = ps_t.tile([P, H], bf16, space="PSUM", tag="aggT")
                    rid = identb[:]
                for mh in range(NH):
                    nc.tensor.matmul(
                        out=htps[:, mh * P:(mh + 1) * P],
                        lhsT=hfm_parts[mh], rhs=rid,
                        is_transpose=True, start=True, stop=True)
                hnm = sb.tile([P, H], bf16, tag="hnm")
                nc.vector.tensor_copy(hnm[:], htps[:])
                r = rows_of(w)
                nc.sync.dma_start(out=hb[w * P:w * P + r, :], in_=hnm[:r, :])
                if pool_into is not None:
                    gtw = sb.tile([P, G], bf16, tag="gtw")
                    nc.sync.dma_start(out=gtw[:],
                                      in_=gT_d[w * P:(w + 1) * P, :])
                    for mh in range(NH):
                        nc.tensor.matmul(
                            out=pool_into[mh],
                            lhsT=hnm[:, mh * P:(mh + 1) * P], rhs=gtw[:],
                            start=(w == 0), stop=(w == NW - 1))

            def emit_ag(l_next, g):
                """AllGather h[l_next%2] group g -> T[l_next%2] segment g."""
                hb = h_ab[l_next % 2]
                Tb = T_ab[l_next % 2]
                nc.gpsimd.collective_compute(
                    "AllGather", mybir.AluOpType.bypass,
                    replica_groups=rg,
                    ins=[hb[g * GROWS:(g + 1) * GROWS, :]],
                    outs=[Tb[g * SEGLEN:(g + 1) * SEGLEN, :]])

            # ---------------- projection (writes h0) ----------------
            wp_sb = wpool.tile([D, H], f32, tag="wp")
            nc.sync.dma_start(out=wp_sb[:], in_=wpT_d[:])
            bp_sb = wpool.tile([P, NH], f32, tag="bp")
            nc.sync.dma_start(out=bp_sb[:], in_=bpT_d[:])
            CW = min(4, WG)
            assert WG % CW == 0
            for wc in range(0, NW, CW):
                cw = CW
                xch = sb.tile([P, CW * P], f32, tag="xch")
                nc.sync.dma_start(out=xch[:, :cw * P],
                                  in_=x_own[:, wc * P:(wc + cw) * P])
                hps = []
                for mh in range(NH):
                    hps_t = ps.tile([P, CW * P], f32, space="PSUM",
                                    tag="mlp", name=f"hps{mh}")
                    nc.tensor.matmul(out=hps_t[:, :cw * P],
                                     lhsT=wp_sb[:, mh * P:(mh + 1) * P],
                                     rhs=xch[:, :cw * P], start=True, stop=True)
                    hps.append(hps_t)
                h0 = []
                for mh in range(NH):
                    h0_t = sb.tile([P, CW * P], f32, tag="h0", name=f"h0{mh}")
                    nc.scalar.activation(
                        h0_t[:, :cw * P], hps[mh][:, :cw * P],
                        mybir.ActivationFunctionType.Relu,
                        bias=bp_sb[:, mh:mh + 1], scale=1.0)
                    h0.append(h0_t)
                for wl in range(cw):
                    w = wc + wl
                    write_node_major(
                        w, [h0[mh][:, wl * P:(wl + 1) * P] for mh in range(NH)],
                        h_ab[0])
                    if (w + 1) % WG == 0:
                        emit_ag(0, w // WG)

            # ---------------- GIN layers ----------------
            ag_queue = []
            pps0 = ppool.tile([P, G], f32, space="PSUM", tag="pps",
                              name="pps0")
            pps1 = ppool.tile([P, G], f32, space="PSUM", tag="pps1",
                              name="pps1")
            pps = [pps0[:], pps1[:]]
            for l in range(L):
                Tb = T_ab[l % 2]
                hb = h_ab[l % 2]
                hn = h_ab[(l + 1) % 2]

                # layer weights
                w1sb = []
                w2sb = []
                for kk in range(NH):
                    t1w = wpool.tile([P, H], bf16, tag=f"w1_{kk}")
                    nc.sync.dma_start(out=t1w[:],
                                      in_=w1T_d[l, kk * P:(kk + 1) * P, :])
                    w1sb.append(t1w)
                    t2w = wpool.tile([P, H], bf16, tag=f"w2_{kk}")
                    nc.sync.dma_start(out=t2w[:],
                                      in_=w2T_d[l, kk * P:(kk + 1) * P, :])
                    w2sb.append(t2w)
                b1sb = wpool.tile([P, NH], f32, tag="b1")
                nc.sync.dma_start(out=b1sb[:], in_=b1T_d[l])
                b2sb = wpool.tile([P, NH], f32, tag="b2")
                nc.sync.dma_start(out=b2sb[:], in_=b2T_d[l])
                ieps = wpool.tile([P, P], bf16, tag="ieps")
                nc.scalar.activation(ieps[:], identb[:],
                                     mybir.ActivationFunctionType.Copy,
                                     bias=0.0, scale=eps1p[:, l:l + 1])

                agg = apool.tile([P, NW * H], bf16, tag="agg")

                def mlp_pair(w0):
                    """GIN MLP for windows w0, w0+1 (node-major agg in SBUF)
                    -> write node-major h into hn."""
                    zTp = sb.tile([P, 2 * H], bf16, tag="zTp")
                    for wl in range(2):
                        w = w0 + wl
                        hw = sb.tile([P, H], bf16, tag="hw")
                        nc.sync.dma_start(out=hw[:],
                                          in_=hb[w * P:(w + 1) * P, :])
                        zps = ps.tile([P, H], f32, space="PSUM", tag="mlp")
                        aggT = ps_t.tile([P, H], bf16, space="PSUM",
                                         tag="aggT")
                        for kk in range(NH):
                            nc.tensor.matmul(
                                out=aggT[:, kk * P:(kk + 1) * P],
                                lhsT=agg[:, w * H + kk * P:
                                         w * H + (kk + 1) * P],
                                rhs=identb[:], is_transpose=True,
                                start=True, stop=True)
                            nc.tensor.matmul(
                                out=zps[:, kk * P:(kk + 1) * P],
                                lhsT=hw[:, kk * P:(kk + 1) * P], rhs=ieps[:],
                                start=True, stop=True)
                        aggTs = sb.tile([P, H], bf16, tag="aggTs")
                        nc.vector.tensor_copy(aggTs[:], aggT[:])
                        zview = zTp[:].rearrange(
                            "p (kk two pp) -> p kk two pp", two=2, pp=P)
                        nc.vector.tensor_add(
                            zview[:, :, wl, :],
                            zps[:].rearrange("p (kk pp) -> p kk pp", pp=P),
                            aggTs[:].rearrange("p (kk pp) -> p kk pp", pp=P))
                    H2 = 2 * H
                    y1ps = ps.tile([P, H2], f32, space="PSUM", tag="mlp")
                    for mh in range(NH):
                        for kk in range(NH):
                            nc.tensor.matmul(
                                out=y1ps[:, mh * 2 * P:(mh + 1) * 2 * P],
                                lhsT=w1sb[kk][:, mh * P:(mh + 1) * P],
                                rhs=zTp[:, kk * 2 * P:(kk + 1) * 2 * P],
                                start=(kk == 0), stop=(kk == NH - 1))
                    y1 = sb.tile([P, H2], bf16, tag="y1")
                    for mh in range(NH):
                        nc.scalar.activation(
                            y1[:, mh * 2 * P:(mh + 1) * 2 * P],
                            y1ps[:, mh * 2 * P:(mh + 1) * 2 * P],
                            mybir.ActivationFunctionType.Relu,
                            bias=b1sb[:, mh:mh + 1], scale=1.0)
                    y2ps = ps.tile([P, H2], f32, space="PSUM", tag="mlp")
                    for mh in range(NH):
                        for kk in range(NH):
                            nc.tensor.matmul(
                                out=y2ps[:, mh * 2 * P:(mh + 1) * 2 * P],
                                lhsT=w2sb[kk][:, mh * P:(mh + 1) * P],
                                rhs=y1[:, kk * 2 * P:(kk + 1) * 2 * P],
                                start=(kk == 0), stop=(kk == NH - 1))
                    h2 = sb.tile([P, H2], bf16, tag="h2")
                    for mh in range(NH):
                        nc.scalar.activation(
                            h2[:, mh * 2 * P:(mh + 1) * 2 * P],
                            y2ps[:, mh * 2 * P:(mh + 1) * 2 * P],
                            mybir.ActivationFunctionType.Relu,
                            bias=b2sb[:, mh:mh + 1], scale=1.0)
                    for wl in range(2):
                        w = w0 + wl
                        write_node_major(
                            w,
                            [h2[:, mh * 2 * P + wl * P:
                                   mh * 2 * P + (wl + 1) * P]
                             for mh in range(NH)],
                            hn, fm_f32=False,
                            pool_into=(pps if l == L - 1 else None))

                # first/last PROCESSED segment per window (seg order is
                # rotated per block so each AG chunk's earliest consumer
                # lands later in the layer)
                first_seg = {}
                last_seg = {}
                for blk in range(NGRP):
                    for s in range(NGRP):
                        for (w, t, first, last) in plan[blk][s]["pieces"]:
                            first_seg.setdefault(w, s)
                            last_seg[w] = s
                for w in range(NW):
                    if w not in first_seg:
                        # no in-edges anywhere: zero the agg slice
                        zagg = sb.tile([P, H], bf16, tag="zagg")
                        nc.gpsimd.memset(zagg[:], 0)
                        nc.vector.tensor_copy(agg[:, w * H:(w + 1) * H],
                                              zagg[:])

                # gather + aggregate: window-block major, segments inner
                next_mlp_w = 0
                done_w = [False] * NW
                for w in range(NW):
                    if w not in last_seg:
                        done_w[w] = True
                for blk in range(NGRP):
                    for si, s in enumerate(range(NGRP)):
                        pl = plan[blk][s]
                        idxt = idxp.tile([P, max(pl["tiles"], 1) * 8],
                                         mybir.dt.int16, tag="idxt")
                        if pl["tiles"]:
                            nc.sync.dma_start(
                                out=idxt[:, :pl["tiles"] * 8],
                                in_=gidx_d[:, pl["t_base"] * 8:
                                           (pl["t_base"] + pl["tiles"]) * 8])
                        npieces = len(pl["pieces"])
                        pieces = pl["pieces"]
                        pi = 0
                        run_ps = None
                        for ci, (t0, t1) in enumerate(pl["calls"]):
                            if si == 1 and ci == 0 and ag_queue:
                                for a in ag_queue:
                                    emit_ag(*a)
                                del ag_queue[:]
                            ct = t1 - t0
                            gb = gbp.tile([P, ct * H], bf16, tag="gbuf")
                            nc.gpsimd.dma_gather(
                                out_ap=gb[:].rearrange(
                                    "p (t d) -> p t d", d=H),
                                in_ap=Tb[s * SEGLEN:(s + 1) * SEGLEN, :],
                                idxs_ap=idxt[:, t0 * 8:t1 * 8],
                                num_idxs=ct * P, num_idxs_reg=ct * P,
                                elem_size=H)
                            pi0 = pi
                            pi1 = pi
                            while pi1 < npieces and pieces[pi1][1] < t1:
                                pi1 += 1
                            cp_n = pi1 - pi0
                            ssb = ssp.tile([P, max(cp_n, 1) * P], bf16,
                                           tag="stile")
                            if cp_n:
                                g0 = pl["p_base"] + pi0
                                nc.sync.dma_start(
                                    out=ssb[:, :cp_n * P].rearrange(
                                        "e (t d) -> e t d", d=P),
                                    in_=s_d[g0:g0 + cp_n].rearrange(
                                        "t e d -> e t d"))
                            while pi < pi1:
                                (w, t, first, last) = pieces[pi]
                                if first:
                                    run_ps = ps_agg.tile(
                                        [P, H], f32, space="PSUM",
                                        tag="aggps")
                                nc.tensor.matmul(
                                    out=run_ps[:],
                                    lhsT=ssb[:, (pi - pi0) * P:
                                             (pi - pi0 + 1) * P],
                                    rhs=gb[:, (t - t0) * H:(t - t0 + 1) * H],
                                    start=first, stop=last)
                                if last:
                                    wsl = agg[:, w * H:(w + 1) * H]
                                    if s == first_seg[w]:
                                        nc.vector.tensor_copy(wsl, run_ps[:])
                                    else:
                                        nc.vector.tensor_add(wsl, wsl,
                                                             run_ps[:])
                                    if s == last_seg[w]:
                                        done_w[w] = True
                                pi += 1
                            # opportunistic MLP pairs (any seg, deferred
                            # across block boundaries to smooth PE load)
                            n_emit = 2
                            while (n_emit and next_mlp_w < NW
                                   and done_w[next_mlp_w]
                                   and done_w[next_mlp_w + 1]):
                                mlp_pair(next_mlp_w)
                                next_mlp_w += 2
                                n_emit -= 1
                                if (next_mlp_w % WG == 0 and l + 1 < L):
                                    ag_queue.append(
                                        (l + 1, next_mlp_w // WG - 1))
                        assert pi == npieces
                # end of layer: drain remaining MLP pairs
                while next_mlp_w < NW:
                    assert done_w[next_mlp_w] and done_w[next_mlp_w + 1]
                    mlp_pair(next_mlp_w)
                    next_mlp_w += 2
                    if (next_mlp_w % WG == 0 and l + 1 < L):
                        ag_queue.append((l + 1, next_mlp_w // WG - 1))

            # ---------------- pooling + head ----------------
            psb = sb.tile([P, NH * G], f32, tag="psb")
            for mh in range(NH):
                nc.vector.tensor_copy(psb[:, mh * G:(mh + 1) * G], pps[mh])
            nc.sync.dma_start(out=pp_in[:], in_=psb[:])
            nc.gpsimd.collective_compute(
                "AllReduce", mybir.AluOpType.add,
                replica_groups=rg, ins=[pp_in[:]], outs=[pp_out[:]])
            ppsb = sb.tile([P, NH * G], f32, tag="ppsb")
            nc.sync.dma_start(out=ppsb[:], in_=pp_out[:])

            cntsb = cpool.tile([P, G], f32)
            nc.sync.dma_start(out=cntsb[:], in_=cnt_d[:])
            cnt2 = cpool.tile([P, G], f32)
            nc.vector.tensor_scalar(out=cnt2[:], in0=cntsb[:], scalar1=1.0,
                                    scalar2=None, op0=mybir.AluOpType.max)
            rec = cpool.tile([P, G], f32)
            nc.vector.reciprocal(rec[:], cnt2[:])
            hg = sb.tile([P, NH * G], f32, tag="hg")
            for mh in range(NH):
                nc.vector.tensor_mul(hg[:, mh * G:(mh + 1) * G],
                                     ppsb[:, mh * G:(mh + 1) * G], rec[:])

            wo1sb = []
            wo2sb = []
            for kk in range(NH):
                t1w = wpool.tile([P, H], f32, tag=f"wo1_{kk}")
                nc.sync.dma_start(out=t1w[:],
                                  in_=wo1T_d[kk * P:(kk + 1) * P, :])
                wo1sb.append(t1w)
                t2w = wpool.tile([P, OUT], f32, tag=f"wo2_{kk}")
                nc.sync.dma_start(out=t2w[:],
                                  in_=wo2T_d[kk * P:(kk + 1) * P, :])
                wo2sb.append(t2w)
            bo1sb = wpool.tile([P, NH], f32, tag="bo1")
            nc.sync.dma_start(out=bo1sb[:], in_=bo1T_d[:])
            bo2sb = wpool.tile([P, NO], f32, tag="bo2")
            nc.sync.dma_start(out=bo2sb[:], in_=bo2T_d[:])

            o1ps = ps.tile([P, NH * G], f32, space="PSUM", tag="mlp")
            for mh in range(NH):
                for kk in range(NH):
                    nc.tensor.matmul(
                        out=o1ps[:, mh * G:(mh + 1) * G],
                        lhsT=wo1sb[kk][:, mh * P:(mh + 1) * P],
                        rhs=hg[:, kk * G:(kk + 1) * G],
                        start=(kk == 0), stop=(kk == NH - 1))
            o1 = sb.tile([P, NH * G], f32, tag="o1")
            for mh in range(NH):
                nc.scalar.activation(
                    o1[:, mh * G:(mh + 1) * G], o1ps[:, mh * G:(mh + 1) * G],
                    mybir.ActivationFunctionType.Relu,
                    bias=bo1sb[:, mh:mh + 1], scale=1.0)
            o2ps = ps.tile([P, NO * G], f32, space="PSUM", tag="mlp")
            for mq in range(NO):
                for kk in range(NH):
                    nc.tensor.matmul(
                        out=o2ps[:, mq * G:(mq + 1) * G],
                        lhsT=wo2sb[kk][:, mq * P:(mq + 1) * P],
                        rhs=o1[:, kk * G:(kk + 1) * G],
                        start=(kk == 0), stop=(kk == NH - 1))
            o2 = sb.tile([P, NO * G], f32, tag="o2")
            for mq in range(NO):
                nc.vector.tensor_scalar_add(
                    o2[:, mq * G:(mq + 1) * G], o2ps[:, mq * G:(mq + 1) * G],
                    bo2sb[:, mq:mq + 1])
            otps = ps.tile([G, OUT], f32, space="PSUM", tag="mlp")
            for mq in range(NO):
                nc.tensor.matmul(
                    out=otps[:, mq * P:(mq + 1) * P],
                    lhsT=o2[:, mq * G:(mq + 1) * G], rhs=ident[:],
                    is_transpose=True, start=True, stop=True)
            osb = sb.tile([G, OUT], f32, tag="osb")
            nc.vector.tensor_copy(osb[:], otps[:])
            nc.sync.dma_start(out=out_d[:], in_=osb[:])

    nc.compile()
    return nc


# --------------------------------------------------------------------------
# public entry
# --------------------------------------------------------------------------

def run(cfg, inputs, mode="hw", trace=False):
    cfg = derive(cfg)
    plan, in_maps = build_host_inputs(cfg, inputs)
    nc = build_program(cfg, plan)
    if mode == "sim":
        from concourse.bass_interp import MultiCoreSim
        sim = MultiCoreSim(nc, num_cores=cfg["NCORES"])
        for cid, core in sim.cores.items():
            for k, v in in_maps[cid].items():
                core.tensor(k)[:] = v
        sim.simulate()
        return np.array(sim.cores[0].mem_tensor("out")), None
    from concourse.bass_utils import run_bass_kernel_spmd
    res = run_bass_kernel_spmd(nc, in_maps, list(range(cfg["NCORES"])),
                               trace=trace)
    return np.asarray(res.results[0]["out"]), res


def kernel(**inputs):
    out, _ = run(full_cfg(), inputs, mode="hw", trace=False)
    return out



# revision 2
# speedup vs baseline: 1.1785x; 1.1785x over previous
"""GIN graph encoder (DispatchGraphEncoder) on 8 Trainium2 NeuronCores. v3.

Gather-desc-roofline design. The dma_gather ucode costs ~8.4ns/descriptor
(byte-independent, measured), so exec time ~= total gather descriptors x
8.4ns + pipeline bubbles. v3 attacks both:

- Bucket-balanced assignment: after degree-LPT, a per-group vector-packing
  pass rebalances dst nodes across (core, window) bins to equalize
  per-(core, seg, window) edge counts across cores (padding ~9% -> ~2%).
- Replicated projection: every core computes the FULL layer-0 table
  locally from a shared permuted x^T (bf16), so layer-0 gathers start
  ~100us in with no AllGather. A small per-core own-projection fills h0.
- Pre-transposed S tiles in DRAM ([128e, p_tot, 128d]) so S loads are
  contiguous per partition (128 descs vs ~2k transposing descs each).
- DMA issue split across engines: gather-critical loads (gidx/S) on Sync,
  MLP h loads/stores + pool on DVE, weights/x/T0/collectives on Act.
  The gather stream never queues behind MLP/projection traffic.
- Cross-layer software pipelining: MLP pairs and next-layer AllGather
  chunks are emitted opportunistically inside the gather-call stream,
  across layer boundaries (no end-of-layer drain except the final one).
"""
import sys

import numpy as np
import ml_dtypes

sys.path.insert(0, "/opt/trn_rl_repo")

from concourse import bass, bacc, mybir, tile  # noqa: E402
from concourse.masks import make_identity  # noqa: E402

P = 128
AG_ENGINE = "gpsimd"  # walrus: CollectiveCompute only on DMA/Pool


def full_cfg():
    return dict(
        N=100000, E=800000, D=128, H=256, OUT=512, L=4, G=64, NCORES=8,
        RN=12500, BLK=14336, NGRP=4, CALL_TILES=8,
    )


def tiny_cfg():
    return dict(
        N=2000, E=8192, D=128, H=256, OUT=512, L=2, G=8, NCORES=8,
        RN=250, BLK=512, NGRP=2, CALL_TILES=4,
    )


def derive(cfg):
    cfg = dict(cfg)
    cfg["NW"] = cfg["BLK"] // P
    assert cfg["NW"] % cfg["NGRP"] == 0
    cfg["WG"] = cfg["NW"] // cfg["NGRP"]          # windows per group
    cfg["GROWS"] = cfg["WG"] * P                  # h rows per group per core
    cfg["SEGLEN"] = cfg["NCORES"] * cfg["GROWS"]  # table rows per segment
    assert cfg["SEGLEN"] <= 32767
    cfg["TROWS"] = cfg["NGRP"] * cfg["SEGLEN"]
    nw = cfg["NW"]
    base, extra = cfg["RN"] // nw, cfg["RN"] % nw
    cfg["SCHED"] = [base + 1 if w < extra else base for w in range(nw)]
    assert max(cfg["SCHED"]) <= P
    return cfg


# --------------------------------------------------------------------------
# host-side preprocessing (pure index/metadata manipulation)
# --------------------------------------------------------------------------

def assign_nodes(cfg, indeg):
    """Degree-balanced LPT: node -> (core, window-slot position)."""
    import heapq
    c = cfg
    nw, ncores = c["NW"], c["NCORES"]
    sched = c["SCHED"]
    heap = []
    for core in range(ncores):
        for w in range(nw):
            heap.append((0.0, core * nw + w))
    heapq.heapify(heap)
    fill = np.zeros(ncores * nw, np.int64)
    n = len(indeg)
    node2core = np.empty(n, np.int64)
    node2pos = np.empty(n, np.int64)
    order = np.argsort(-indeg, kind="stable")
    for v in order:
        while True:
            load, b = heapq.heappop(heap)
            w = b % nw
            if fill[b] < sched[w]:
                break
        node2core[v] = b // nw
        node2pos[v] = w * P + fill[b]
        fill[b] += 1
        if fill[b] < sched[w]:
            heapq.heappush(heap, (load + float(indeg[v]), b))
    return node2core, node2pos


def rebalance_buckets(cfg, src, dst, node2core, node2pos):
    """Within each src-group, reassign dst nodes to (core, window) bins to
    equalize per-(core, s, w) in-edge counts across cores (minimizes the
    cross-core max padding in the uniform gather stream). Group membership
    (node2pos // (WG*P) // ...) is preserved so src-side vectors stay fixed.
    """
    c = cfg
    nw, ncores, ngrp, wg = c["NW"], c["NCORES"], c["NGRP"], c["WG"]
    sched = np.asarray(c["SCHED"], np.int64)
    w_of = node2pos // P
    grp_of = w_of // wg
    sgrp = grp_of[src]                       # src group per edge

    # per-dst-node 4-vector of in-edge counts by src group
    vmat = np.zeros((c["N"], ngrp), np.int64)
    np.add.at(vmat, (dst, sgrp), 1)

    new_core = node2core.copy()
    new_pos = node2pos.copy()
    for g in range(ngrp):
        nodes = np.flatnonzero(grp_of == g)
        v = vmat[nodes]                       # [M, ngrp]
        order = np.argsort(-v.sum(axis=1), kind="stable")
        wlo = g * wg
        cap = np.tile(sched[wlo:wlo + wg], (ncores, 1)).astype(np.int64)
        cnt = np.zeros((ncores, ngrp, wg), np.int64)
        fill = np.zeros((ncores, wg), np.int64)
        u = np.zeros((ngrp, wg), np.int64)
        for idx in order:
            node = nodes[idx]
            vv = v[idx]                        # [ngrp]
            # delta objective for adding vv to each (core, w):
            # sum_s max(cnt[c,s,w]+vv[s], u[s,w]) - u[s,w]
            nc_ = cnt + vv[None, :, None]
            delta = np.maximum(nc_ - u[None, :, :], 0).sum(axis=1)  # [nc, wg]
            delta = np.where(fill < cap, delta, 10**9)
            # tie-break on lowest fill to keep capacities comfortable
            flat = np.argmin(delta * 10**6 + fill, axis=None)
            ci, wi = np.unravel_index(flat, delta.shape)
            cnt[ci, :, wi] += vv
            u[:, wi] = np.maximum(u[:, wi], cnt[ci, :, wi])
            new_core[node] = ci
            new_pos[node] = (wlo + wi) * P + fill[ci, wi]
            fill[ci, wi] += 1
        assert (fill == cap).all()
    return new_core, new_pos


def preprocess(cfg, edge_index):
    """Build the uniform tile/piece program + per-core gather/S data."""
    c = cfg
    src = np.asarray(edge_index[0], dtype=np.int64)
    dst = np.asarray(edge_index[1], dtype=np.int64)

    indeg = np.bincount(dst, minlength=c["N"])
    node2core, node2pos = assign_nodes(c, indeg)
    node2core, node2pos = rebalance_buckets(c, src, dst, node2core, node2pos)

    ncores, ngrp, nw, wg = c["NCORES"], c["NGRP"], c["NW"], c["WG"]
    seglen, grows = c["SEGLEN"], c["GROWS"]

    w_of = node2pos // P
    grp_of = w_of // wg
    rel_of = node2core * grows + (w_of % wg) * P + (node2pos % P)

    owner = node2core[dst]
    s_e = grp_of[src]
    rel_e = rel_of[src]
    dw_e = node2pos[dst] // P
    dc_e = node2pos[dst] % P

    counts = np.zeros((ncores, ngrp, nw), np.int64)
    np.add.at(counts, (owner, s_e, dw_e), 1)
    u = counts.max(axis=0)                        # [ngrp, nw] uniform counts

    order = np.lexsort((dw_e, s_e, owner))
    o_s, s_s, r_s, w_s, c_s = (owner[order], s_e[order], rel_e[order],
                               dw_e[order], dc_e[order])

    key = (o_s * ngrp + s_s) * nw + w_s
    bounds = np.flatnonzero(np.diff(key)) + 1
    starts = np.concatenate(([0], bounds))
    ends = np.concatenate((bounds, [len(key)]))
    range_of = {}
    for a, b in zip(starts, ends):
        range_of[(int(o_s[a]), int(s_s[a]), int(w_s[a]))] = (int(a), int(b))

    plan = [[None] * ngrp for _ in range(ngrp)]   # [block][seg]
    t_tot = 0
    p_tot = 0
    for blk in range(ngrp):
        wlo, whi = blk * wg, (blk + 1) * wg
        for s in range(ngrp):
            us = u[s, wlo:whi]
            off = np.concatenate(([0], np.cumsum(us)))
            length = int(off[-1])
            tiles = (length + P - 1) // P
            pieces = []
            for wi in range(wg):
                a, b = int(off[wi]), int(off[wi + 1])
                if a == b:
                    continue
                ta, tb = a // P, (b - 1) // P
                for t in range(ta, tb + 1):
                    pieces.append((wlo + wi, t, a >= t * P,
                                   b <= (t + 1) * P))
            calls = []
            t0 = 0
            while t0 < tiles:
                t1 = min(t0 + c["CALL_TILES"], tiles)
                calls.append((t0, t1))
                t0 = t1
            plan[blk][s] = dict(tiles=tiles, calls=calls, pieces=pieces,
                                off=off, t_base=t_tot, p_base=p_tot)
            t_tot += tiles
            p_tot += len(pieces)

    gidx = np.zeros((ncores, P, t_tot * 8), np.int16)
    s_T = np.zeros((ncores, P, p_tot, P), ml_dtypes.bfloat16)

    for blk in range(ngrp):
        wlo = blk * wg
        for s in range(ngrp):
            pl = plan[blk][s]
            off = pl["off"]
            t_base, p_base = pl["t_base"], pl["p_base"]
            for core in range(ncores):
                stream_rel = np.zeros(pl["tiles"] * P, np.int64)
                stream_col = np.full(pl["tiles"] * P, -1, np.int64)
                for wi in range(c["WG"]):
                    a = int(off[wi])
                    rng = range_of.get((core, s, wlo + wi))
                    if rng is None:
                        continue
                    ea, eb = rng
                    n = eb - ea
                    stream_rel[a:a + n] = r_s[ea:eb]
                    stream_col[a:a + n] = c_s[ea:eb]
                jj = np.arange(pl["tiles"] * P)
                tt = jj // P
                within = jj % P
                cols = (t_base + tt) * 8 + within // 16
                rows = within % 16
                for repl in range(8):
                    gidx[core, rows + 16 * repl, cols] = (
                        stream_rel.astype(np.int16))
                for i, (w, t, first, last) in enumerate(pl["pieces"]):
                    wi = w - wlo
                    a = max(int(off[wi]), t * P)
                    b = min(int(off[wi + 1]), (t + 1) * P)
                    lo = a - t * P
                    hi = b - t * P
                    sl = stream_col[t * P + lo:t * P + hi]
                    rws = np.arange(lo, hi)
                    m = sl >= 0
                    s_T[core, rws[m], p_base + i, sl[m]] = 1

    return plan, gidx, s_T, node2core, node2pos


def build_host_inputs(cfg, inputs):
    c = cfg
    x = np.asarray(inputs["x"], np.float32)
    batch = np.asarray(inputs["batch"], np.int64)
    plan, gidx, s_T, node2core, node2pos = preprocess(c, inputs["edge_index"])

    L, H, D, OUT, G = c["L"], c["H"], c["D"], c["OUT"], c["G"]
    node_w = np.asarray(inputs["node_w"], np.float32)
    node_b = np.asarray(inputs["node_b"], np.float32)
    gw1 = np.asarray(inputs["gin_w1"], np.float32)
    gb1 = np.asarray(inputs["gin_b1"], np.float32)
    gw2 = np.asarray(inputs["gin_w2"], np.float32)
    gb2 = np.asarray(inputs["gin_b2"], np.float32)
    eps = np.asarray(inputs["eps"], np.float32)
    ow1 = np.asarray(inputs["out_w1"], np.float32)
    ob1 = np.asarray(inputs["out_b1"], np.float32)
    ow2 = np.asarray(inputs["out_w2"], np.float32)
    ob2 = np.asarray(inputs["out_b2"], np.float32)

    cnt = np.bincount(batch, minlength=G).astype(np.float32)

    # permuted full x^T in table order (shared by all cores)
    w_of = node2pos // P
    grp_of = w_of // c["WG"]
    rel_of = (node2core * c["GROWS"] + (w_of % c["WG"]) * P
              + (node2pos % P))
    grow = grp_of * c["SEGLEN"] + rel_of
    xTp = np.zeros((D, c["TROWS"]), ml_dtypes.bfloat16)
    xTp[:, grow] = x.T

    common = {
        "xTp": xTp,
        "wpT": np.ascontiguousarray(node_w.T).astype(ml_dtypes.bfloat16),
        "bpT": np.ascontiguousarray(node_b.reshape(H // P, P).T),  # [P, H/P]
        "w1T": np.ascontiguousarray(
            np.transpose(gw1, (0, 2, 1))).astype(ml_dtypes.bfloat16),
        "b1T": np.ascontiguousarray(np.transpose(
            gb1.reshape(L, H // P, P), (0, 2, 1))),          # [L, P, H/P]
        "w2T": np.ascontiguousarray(
            np.transpose(gw2, (0, 2, 1))).astype(ml_dtypes.bfloat16),
        "b2T": np.ascontiguousarray(np.transpose(
            gb2.reshape(L, H // P, P), (0, 2, 1))),
        "eps_rep": np.tile(eps.reshape(1, L), (P, 1)).astype(np.float32),
        "wo1T": np.ascontiguousarray(ow1.T),                # [H, H]
        "bo1T": np.ascontiguousarray(ob1.reshape(H // P, P).T),
        "wo2T": np.ascontiguousarray(ow2.T),                # [H, OUT]
        "bo2T": np.ascontiguousarray(ob2.reshape(OUT // P, P).T),
        "cnt_rep": np.tile(cnt.reshape(1, G), (P, 1)),
    }

    in_maps = []
    for core in range(c["NCORES"]):
        mine = np.flatnonzero(node2core == core)
        pos = node2pos[mine]
        xo = np.zeros((D, c["BLK"]), ml_dtypes.bfloat16)
        xo[:, pos] = x[mine].T
        gT = np.zeros((c["BLK"], G), ml_dtypes.bfloat16)
        gT[pos, batch[mine]] = 1.0
        m = dict(common)
        m["x_own"] = xo
        m["gidx"] = gidx[core]
        m["s_T"] = s_T[core]
        m["gT"] = gT
        in_maps.append(m)
    return plan, in_maps


# --------------------------------------------------------------------------
# device program
# --------------------------------------------------------------------------

def build_program(cfg, plan):
    c = cfg
    L, H, D, OUT, G = c["L"], c["H"], c["D"], c["OUT"], c["G"]
    NW, BLK, NGRP, WG = c["NW"], c["BLK"], c["NGRP"], c["WG"]
    GROWS, SEGLEN, TROWS = c["GROWS"], c["SEGLEN"], c["TROWS"]
    NH = H // P
    NO = OUT // P
    f32 = mybir.dt.float32
    bf16 = mybir.dt.bfloat16
    t_tot = sum(pl["tiles"] for row in plan for pl in row)
    p_tot = sum(len(pl["pieces"]) for row in plan for pl in row)

    nc = bacc.Bacc("TRN2", target_bir_lowering=False, debug=False)

    xTp_d = nc.dram_tensor("xTp", [D, TROWS], bf16, kind="ExternalInput")
    x_own = nc.dram_tensor("x_own", [D, BLK], bf16, kind="ExternalInput")
    gidx_d = nc.dram_tensor("gidx", [P, t_tot * 8], mybir.dt.int16,
                            kind="ExternalInput")
    s_d = nc.dram_tensor("s_T", [P, p_tot, P], bf16, kind="ExternalInput")
    gT_d = nc.dram_tensor("gT", [BLK, G], bf16, kind="ExternalInput")
    wpT_d = nc.dram_tensor("wpT", [D, H], bf16, kind="ExternalInput")
    bpT_d = nc.dram_tensor("bpT", [P, NH], f32, kind="ExternalInput")
    w1T_d = nc.dram_tensor("w1T", [L, H, H], bf16, kind="ExternalInput")
    b1T_d = nc.dram_tensor("b1T", [L, P, NH], f32, kind="ExternalInput")
    w2T_d = nc.dram_tensor("w2T", [L, H, H], bf16, kind="ExternalInput")
    b2T_d = nc.dram_tensor("b2T", [L, P, NH], f32, kind="ExternalInput")
    eps_d = nc.dram_tensor("eps_rep", [P, L], f32, kind="ExternalInput")
    wo1T_d = nc.dram_tensor("wo1T", [H, H], f32, kind="ExternalInput")
    bo1T_d = nc.dram_tensor("bo1T", [P, NH], f32, kind="ExternalInput")
    wo2T_d = nc.dram_tensor("wo2T", [H, OUT], f32, kind="ExternalInput")
    bo2T_d = nc.dram_tensor("bo2T", [P, NO], f32, kind="ExternalInput")
    cnt_d = nc.dram_tensor("cnt_rep", [P, G], f32, kind="ExternalInput")

    out_d = nc.dram_tensor("out", [G, OUT], f32, kind="ExternalOutput")

    h_ab = [nc.dram_tensor(f"h{i}", [BLK, H], bf16) for i in range(2)]
    T_ab = [nc.dram_tensor(f"T{i}", [TROWS, H], bf16, addr_space="Shared")
            for i in range(2)]
    pp_in = nc.dram_tensor("pp_in", [P, NH * G], f32)
    pp_out = nc.dram_tensor("pp_out", [P, NH * G], f32, addr_space="Shared")

    rg = [list(range(c["NCORES"]))]

    def emit_collective(kind, op, ins, outs):
        eng = nc.scalar if AG_ENGINE == "scalar" else nc.gpsimd
        bass.BassGpSimd.collective_compute(
            eng, kind, op, replica_groups=rg, ins=ins, outs=outs)

    with tile.TileContext(nc) as tc:
        with (
            tc.tile_pool(name="const", bufs=1) as cpool,
            tc.tile_pool(name="head", bufs=1) as hpool,
            tc.tile_pool(name="agg", bufs=1) as apool,
            tc.tile_pool(name="wt", bufs=2) as wpool,
            tc.tile_pool(name="sb", bufs=4) as sb,
            tc.tile_pool(name="idx", bufs=3) as idxp,
            tc.tile_pool(name="gb", bufs=6) as gbp,
            tc.tile_pool(name="ssb", bufs=6) as ssp,
            tc.tile_pool(name="ps", bufs=4, space="PSUM") as ps,
            tc.tile_pool(name="ps_t", bufs=2, space="PSUM") as ps_t,
            tc.tile_pool(name="ps_agg", bufs=2, space="PSUM") as ps_agg,
        ):
            ident = cpool.tile([P, P], f32)
            make_identity(nc, ident[:])
            identb = cpool.tile([P, P], bf16)
            nc.vector.tensor_copy(identb[:], ident[:])
            eps_t = cpool.tile([P, L], f32)
            nc.scalar.dma_start(out=eps_t[:], in_=eps_d[:])
            eps1p = cpool.tile([P, L], f32)
            nc.scalar.add(eps1p[:], eps_t[:], 1.0)

            # head weights preloaded up front (consumed only at the tail)
            wo1sb = []
            wo2sb = []
            for kk in range(NH):
                t1w = hpool.tile([P, H], f32, name=f"wo1_{kk}")
                nc.scalar.dma_start(out=t1w[:],
                                    in_=wo1T_d[kk * P:(kk + 1) * P, :])
                wo1sb.append(t1w)
                t2w = hpool.tile([P, OUT], f32, name=f"wo2_{kk}")
                nc.scalar.dma_start(out=t2w[:],
                                    in_=wo2T_d[kk * P:(kk + 1) * P, :])
                wo2sb.append(t2w)
            bo1sb = hpool.tile([P, NH], f32, name="bo1")
            nc.scalar.dma_start(out=bo1sb[:], in_=bo1T_d[:])
            bo2sb = hpool.tile([P, NO], f32, name="bo2")
            nc.scalar.dma_start(out=bo2sb[:], in_=bo2T_d[:])
            cntsb = cpool.tile([P, G], f32)
            nc.scalar.dma_start(out=cntsb[:], in_=cnt_d[:])

            # zero both h buffers once (pad slots inside every window)
            ZC = min(8, NW)
            zt = cpool.tile([P, ZC * H], bf16)
            nc.vector.memset(zt[:], 0)
            assert BLK % (ZC * P) == 0
            for hb in h_ab:
                for zb in range(BLK // (ZC * P)):
                    nc.sync.dma_start(
                        out=hb[zb * ZC * P:(zb + 1) * ZC * P, :].rearrange(
                            "(a p) c -> p a c", p=P),
                        in_=zt[:].rearrange("p (a c) -> p a c", c=H))

            # ------------- projection (replicated table + own h0) -------------
            wp_sb = cpool.tile([D, H], bf16)
            nc.scalar.dma_start(out=wp_sb[:], in_=wpT_d[:])
            bp_sb = cpool.tile([P, NH], f32)
            nc.scalar.dma_start(out=bp_sb[:], in_=bpT_d[:])

            CW = min(4, WG)
            assert WG % CW == 0

            def proj_chunk(src_d, col0, cw, dst_d, drow0):
                """project cw windows of src_d cols [col0, col0+cw*P) and
                store node-major into dst_d rows [drow0, drow0+cw*P)."""
                xch = sb.tile([P, CW * P], bf16, tag="xch")
                nc.sync.dma_start(out=xch[:, :cw * P],
                                  in_=src_d[:, col0:col0 + cw * P])
                h0 = []
                for mh in range(NH):
                    hps = ps.tile([P, CW * P], f32, space="PSUM", tag="mlp",
                                  name=f"hps{mh}")
                    nc.tensor.matmul(out=hps[:, :cw * P],
                                     lhsT=wp_sb[:, mh * P:(mh + 1) * P],
                                     rhs=xch[:, :cw * P], start=True,
                                     stop=True)
                    h0t = sb.tile([P, CW * P], bf16, tag=f"ph{mh}")
                    nc.vector.tensor_scalar(
                        out=h0t[:, :cw * P], in0=hps[:, :cw * P],
                        scalar1=bp_sb[:, mh:mh + 1], scalar2=0.0,
                        op0=mybir.AluOpType.add, op1=mybir.AluOpType.max)
                    h0.append(h0t)
                hnm4 = sb.tile([P, CW * H], bf16, tag="hnm4")
                for wl in range(cw):
                    htps = ps_t.tile([P, H], bf16, space="PSUM", tag="aggT")
                    for mh in range(NH):
                        nc.tensor.matmul(
                            out=htps[:, mh * P:(mh + 1) * P],
                            lhsT=h0[mh][:, wl * P:(wl + 1) * P], rhs=identb[:],
                            is_transpose=True, start=True, stop=True)
                    nc.vector.tensor_copy(hnm4[:, wl * H:(wl + 1) * H],
                                          htps[:])
                nc.scalar.dma_start(
                    out=dst_d[drow0:drow0 + cw * P, :].rearrange(
                        "(a p) c -> p a c", p=P),
                    in_=hnm4[:, :cw * H].rearrange("p (a c) -> p a c", c=H))

            # table proj emission is interleaved with the gather stream via
            # a cursor the pipeline advances ahead of each segment's use.
            proj_state = dict(next_chunk=0)
            tw_total = TROWS // P
            chunks_per_seg = SEGLEN // (CW * P)

            def pump_proj(until_chunk):
                while (proj_state["next_chunk"] < until_chunk
                       and proj_state["next_chunk"] * CW < tw_total):
                    ci = proj_state["next_chunk"]
                    proj_chunk(xTp_d, ci * CW * P, CW, T_ab[0], ci * CW * P)
                    proj_state["next_chunk"] += 1

            # segment 0 of the table first, then own h0, then the pipeline
            # pulls the rest ahead of the gather stream.
            pump_proj(chunks_per_seg)
            for wc in range(0, NW, CW):
                proj_chunk(x_own, wc * P, CW, h_ab[0], wc * P)

            # ---------------- pipelined GIN layers ----------------
            sched = c["SCHED"]
            agg = apool.tile([P, NW * H], bf16)

            pacc = cpool.tile([P, NH * G], f32)
            nc.vector.memset(pacc[:], 0)

            # per-(blk, s) first/last segment per window (same for all layers)
            first_seg = {}
            last_seg = {}
            for blk in range(NGRP):
                for s in range(NGRP):
                    for (w, t, first, last) in plan[blk][s]["pieces"]:
                        first_seg.setdefault(w, s)
                        last_seg[w] = s

            ctx = [None] * L
            ready = []            # FIFO of (l, w0) MLP pairs
            ag_pending = []       # (l, g) AGs whose MLP pairs are emitted
            emitted_ags = set()   # (l, g) AllGather chunks emitted

            def load_layer(l):
                w1sb = []
                w2sb = []
                for kk in range(NH):
                    t1w = wpool.tile([P, H], bf16, tag=f"w1_{kk}")
                    nc.scalar.dma_start(out=t1w[:],
                                        in_=w1T_d[l, kk * P:(kk + 1) * P, :])
                    w1sb.append(t1w)
                    t2w = wpool.tile([P, H], bf16, tag=f"w2_{kk}")
                    nc.scalar.dma_start(out=t2w[:],
                                        in_=w2T_d[l, kk * P:(kk + 1) * P, :])
                    w2sb.append(t2w)
                b1sb = wpool.tile([P, NH], f32, tag="b1")
                nc.scalar.dma_start(out=b1sb[:], in_=b1T_d[l])
                b2sb = wpool.tile([P, NH], f32, tag="b2")
                nc.scalar.dma_start(out=b2sb[:], in_=b2T_d[l])
                ieps = wpool.tile([P, P], bf16, tag="ieps")
                nc.scalar.activation(ieps[:], identb[:],
                                     mybir.ActivationFunctionType.Copy,
                                     bias=0.0, scale=eps1p[:, l:l + 1])
                done_w = [w not in last_seg for w in range(NW)]
                ctx[l] = dict(w1=w1sb, w2=w2sb, b1=b1sb, b2=b2sb, ieps=ieps,
                              done=done_w, next_w=0, emit_w=0)

            def emit_ag(l_next, g):
                hb = h_ab[l_next % 2]
                Tb = T_ab[l_next % 2]
                emit_collective(
                    "AllGather", mybir.AluOpType.bypass,
                    ins=[hb[g * GROWS:(g + 1) * GROWS, :]],
                    outs=[Tb[g * SEGLEN:(g + 1) * SEGLEN, :]])
                emitted_ags.add((l_next, g))

            def write_pair(l, w0, hfm_cols):
                """hfm_cols(mh, wl) -> [P, P] AP feature-major; transpose
                both windows -> node-major, one store, optional pooling."""
                hb = h_ab[(l + 1) % 2]
                hnm2 = sb.tile([P, 2 * H], bf16, tag="hnm2")
                for wl in range(2):
                    htps = ps_t.tile([P, H], bf16, space="PSUM", tag="aggT")
                    for mh in range(NH):
                        nc.tensor.matmul(
                            out=htps[:, mh * P:(mh + 1) * P],
                            lhsT=hfm_cols(mh, wl), rhs=identb[:],
                            is_transpose=True, start=True, stop=True)
                    nc.vector.tensor_copy(hnm2[:, wl * H:(wl + 1) * H],
                                          htps[:])
                nc.sync.dma_start(
                    out=hb[w0 * P:(w0 + 2) * P, :].rearrange(
                        "(a p) c -> p a c", p=P),
                    in_=hnm2[:].rearrange("p (a c) -> p a c", c=H))
                if l == L - 1:
                    for wl in range(2):
                        w = w0 + wl
                        gtw = sb.tile([P, G], bf16, tag="gtw")
                        nc.sync.dma_start(out=gtw[:],
                                          in_=gT_d[w * P:(w + 1) * P, :])
                        ppw = ps.tile([P, NH * G], f32, space="PSUM",
                                      tag="mlp")
                        for mh in range(NH):
                            nc.tensor.matmul(
                                out=ppw[:, mh * G:(mh + 1) * G],
                                lhsT=hnm2[:, wl * H + mh * P:
                                          wl * H + (mh + 1) * P],
                                rhs=gtw[:], start=True, stop=True)
                        nc.vector.tensor_add(pacc[:], pacc[:], ppw[:])

            def mlp_pair(l, w0):
                st = ctx[l]
                hb = h_ab[l % 2]
                zTp = sb.tile([P, 2 * H], bf16, tag="zTp")
                for wl in range(2):
                    w = w0 + wl
                    hw = sb.tile([P, H], bf16, tag="hw")
                    nc.sync.dma_start(out=hw[:],
                                        in_=hb[w * P:(w + 1) * P, :])
                    zps = ps.tile([P, H], f32, space="PSUM", tag="mlp")
                    aggT = ps_t.tile([P, H], bf16, space="PSUM", tag="aggT")
                    for kk in range(NH):
                        nc.tensor.matmul(
                            out=aggT[:, kk * P:(kk + 1) * P],
                            lhsT=agg[:, w * H + kk * P:w * H + (kk + 1) * P],
                            rhs=identb[:], is_transpose=True,
                            start=True, stop=True)
                        nc.tensor.matmul(
                            out=zps[:, kk * P:(kk + 1) * P],
                            lhsT=hw[:, kk * P:(kk + 1) * P], rhs=st["ieps"][:],
                            start=True, stop=True)
                    aggTs = sb.tile([P, H], bf16, tag="aggTs")
                    nc.vector.tensor_copy(aggTs[:], aggT[:])
                    zview = zTp[:].rearrange(
                        "p (kk two pp) -> p kk two pp", two=2, pp=P)
                    nc.vector.tensor_add(
                        zview[:, :, wl, :],
                        zps[:].rearrange("p (kk pp) -> p kk pp", pp=P),
                        aggTs[:].rearrange("p (kk pp) -> p kk pp", pp=P))
                H2 = 2 * H
                y1ps = ps.tile([P, H2], f32, space="PSUM", tag="mlp")
                for mh in range(NH):
                    for kk in range(NH):
                        nc.tensor.matmul(
                            out=y1ps[:, mh * 2 * P:(mh + 1) * 2 * P],
                            lhsT=st["w1"][kk][:, mh * P:(mh + 1) * P],
                            rhs=zTp[:, kk * 2 * P:(kk + 1) * 2 * P],
                            start=(kk == 0), stop=(kk == NH - 1))
                y1 = sb.tile([P, H2], bf16, tag="y1")
                for mh in range(NH):
                    nc.scalar.activation(
                        y1[:, mh * 2 * P:(mh + 1) * 2 * P],
                        y1ps[:, mh * 2 * P:(mh + 1) * 2 * P],
                        mybir.ActivationFunctionType.Relu,
                        bias=st["b1"][:, mh:mh + 1], scale=1.0)
                y2ps = ps.tile([P, H2], f32, space="PSUM", tag="mlp")
                for mh in range(NH):
                    for kk in range(NH):
                        nc.tensor.matmul(
                            out=y2ps[:, mh * 2 * P:(mh + 1) * 2 * P],
                            lhsT=st["w2"][kk][:, mh * P:(mh + 1) * P],
                            rhs=y1[:, kk * 2 * P:(kk + 1) * 2 * P],
                            start=(kk == 0), stop=(kk == NH - 1))
                h2 = sb.tile([P, H2], bf16, tag="h2")
                for mh in range(NH):
                    nc.scalar.activation(
                        h2[:, mh * 2 * P:(mh + 1) * 2 * P],
                        y2ps[:, mh * 2 * P:(mh + 1) * 2 * P],
                        mybir.ActivationFunctionType.Relu,
                        bias=st["b2"][:, mh:mh + 1], scale=1.0)
                write_pair(l, w0, lambda mh, wl: h2[:, mh * 2 * P + wl * P:
                                                    mh * 2 * P + (wl + 1) * P])

            def note_done(l, w):
                st = ctx[l]
                st["done"][w] = True
                while (st["next_w"] < NW and st["done"][st["next_w"]]
                       and st["done"][st["next_w"] + 1]):
                    ready.append((l, st["next_w"]))
                    st["next_w"] += 2

            def pump(n):
                while n > 0 and ready:
                    l, w0 = ready.pop(0)
                    mlp_pair(l, w0)
                    st = ctx[l]
                    st["emit_w"] = w0 + 2
                    if st["emit_w"] % WG == 0 and l + 1 < L:
                        ag_pending.append((l + 1, st["emit_w"] // WG - 1))
                    n -= 1

            def flush_ags(step=None):
                for ent in list(ag_pending):
                    al, g = ent
                    cons = al * NGRP * NGRP + g   # step of (al, blk0, s=g)
                    if step is None or cons <= step + 1:
                        ag_pending.remove(ent)
                        emit_ag(al, g)

            def ensure_ag(l, s):
                if l == 0:
                    return
                while (l, s) not in emitted_ags and (l, s) not in ag_pending:
                    assert ready, f"pipeline deadlock at layer {l} seg {s}"
                    pump(1)
                if (l, s) not in emitted_ags:
                    flush_ags()

            for l in range(L):
                load_layer(l)
                Tb = T_ab[l % 2]
                for w in range(NW):
                    if w not in first_seg:
                        zagg = sb.tile([P, H], bf16, tag="zagg")
                        nc.vector.memset(zagg[:], 0)
                        nc.vector.tensor_copy(agg[:, w * H:(w + 1) * H],
                                              zagg[:])
                for blk in range(NGRP):
                    for s in range(NGRP):
                        flush_ags((l * NGRP + blk) * NGRP + s)
                        ensure_ag(l, s)
                        # all of segment s's table writes must be emitted
                        # before its layer-0 gathers (Tile deps need the
                        # writes to exist first)
                        if l == 0:
                            pump_proj((s + 1) * chunks_per_seg)
                        pl = plan[blk][s]
                        idxt = idxp.tile([P, max(pl["tiles"], 1) * 8],
                                         mybir.dt.int16, tag="idxt")
                        if pl["tiles"]:
                            nc.sync.dma_start(
                                out=idxt[:, :pl["tiles"] * 8],
                                in_=gidx_d[:, pl["t_base"] * 8:
                                           (pl["t_base"] + pl["tiles"]) * 8])
                        npieces = len(pl["pieces"])
                        pieces = pl["pieces"]
                        pi = 0
                        run_ps = None
                        for (t0, t1) in pl["calls"]:
                            ct = t1 - t0
                            gb = gbp.tile([P, ct * H], bf16, tag="gbuf")
                            nc.gpsimd.dma_gather(
                                out_ap=gb[:].rearrange(
                                    "p (t d) -> p t d", d=H),
                                in_ap=Tb[s * SEGLEN:(s + 1) * SEGLEN, :],
                                idxs_ap=idxt[:, t0 * 8:t1 * 8],
                                num_idxs=ct * P, num_idxs_reg=ct * P,
                                elem_size=H)
                            pi0 = pi
                            pi1 = pi
                            while pi1 < npieces and pieces[pi1][1] < t1:
                                pi1 += 1
                            cp_n = pi1 - pi0
                            ssb = ssp.tile([P, max(cp_n, 1) * P], bf16,
                                           tag="stile")
                            if cp_n:
                                g0 = pl["p_base"] + pi0
                                nc.sync.dma_start(
                                    out=ssb[:, :cp_n * P].rearrange(
                                        "e (t d) -> e t d", d=P),
                                    in_=s_d[:, g0:g0 + cp_n, :])
                            while pi < pi1:
                                (w, t, first, last) = pieces[pi]
                                if first:
                                    run_ps = ps_agg.tile(
                                        [P, H], f32, space="PSUM",
                                        tag="aggps")
                                nc.tensor.matmul(
                                    out=run_ps[:],
                                    lhsT=ssb[:, (pi - pi0) * P:
                                             (pi - pi0 + 1) * P],
                                    rhs=gb[:, (t - t0) * H:(t - t0 + 1) * H],
                                    start=first, stop=last)
                                if last:
                                    wsl = agg[:, w * H:(w + 1) * H]
                                    if s == first_seg[w]:
                                        nc.vector.tensor_copy(wsl, run_ps[:])
                                    else:
                                        nc.vector.tensor_add(wsl, wsl,
                                                             run_ps[:])
                                    if s == last_seg[w]:
                                        note_done(l, w)
                                pi += 1
                            pump(2)
                        assert pi == npieces
            # final drain
            pump(10**9)
            flush_ags()
            assert not ready

            # ---------------- pooling + head ----------------
            nc.sync.dma_start(out=pp_in[:], in_=pacc[:])
            emit_collective("AllReduce", mybir.AluOpType.add,
                            ins=[pp_in[:]], outs=[pp_out[:]])
            ppsb = sb.tile([P, NH * G], f32, tag="ppsb")
            nc.sync.dma_start(out=ppsb[:], in_=pp_out[:])

            cnt2 = cpool.tile([P, G], f32)
            nc.vector.tensor_scalar(out=cnt2[:], in0=cntsb[:], scalar1=1.0,
                                    scalar2=None, op0=mybir.AluOpType.max)
            rec = cpool.tile([P, G], f32)
            nc.vector.reciprocal(rec[:], cnt2[:])
            hg = sb.tile([P, NH * G], f32, tag="hg")
            for mh in range(NH):
                nc.vector.tensor_mul(hg[:, mh * G:(mh + 1) * G],
                                     ppsb[:, mh * G:(mh + 1) * G], rec[:])

            o1ps = ps.tile([P, NH * G], f32, space="PSUM", tag="mlp")
            for mh in range(NH):
                for kk in range(NH):
                    nc.tensor.matmul(
                        out=o1ps[:, mh * G:(mh + 1) * G],
                        lhsT=wo1sb[kk][:, mh * P:(mh + 1) * P],
                        rhs=hg[:, kk * G:(kk + 1) * G],
                        start=(kk == 0), stop=(kk == NH - 1))
            o1 = sb.tile([P, NH * G], f32, tag="o1")
            for mh in range(NH):
                nc.scalar.activation(
                    o1[:, mh * G:(mh + 1) * G], o1ps[:, mh * G:(mh + 1) * G],
                    mybir.ActivationFunctionType.Relu,
                    bias=bo1sb[:, mh:mh + 1], scale=1.0)
            o2ps = ps.tile([P, NO * G], f32, space="PSUM", tag="mlp")
            for mq in range(NO):
                for kk in range(NH):
                    nc.tensor.matmul(
                        out=o2ps[:, mq * G:(mq + 1) * G],
                        lhsT=wo2sb[kk][:, mq * P:(mq + 1) * P],
                        rhs=o1[:, kk * G:(kk + 1) * G],
                        start=(kk == 0), stop=(kk == NH - 1))
            o2 = sb.tile([P, NO * G], f32, tag="o2")
            for mq in range(NO):
                nc.vector.tensor_scalar_add(
                    o2[:, mq * G:(mq + 1) * G], o2ps[:, mq * G:(mq + 1) * G],
                    bo2sb[:, mq:mq + 1])
            otps = ps.tile([G, OUT], f32, space="PSUM", tag="mlp")
            for mq in range(NO):
                nc.tensor.matmul(
                    out=otps[:, mq * P:(mq + 1) * P],
                    lhsT=o2[:, mq * G:(mq + 1) * G], rhs=ident[:],
                    is_transpose=True, start=True, stop=True)
            osb = sb.tile([G, OUT], f32, tag="osb")
            nc.vector.tensor_copy(osb[:], otps[:])
            nc.sync.dma_start(out=out_d[:], in_=osb[:])

    nc.compile()
    return nc


# --------------------------------------------------------------------------
# public entry
# --------------------------------------------------------------------------

def run(cfg, inputs, mode="hw", trace=False):
    cfg = derive(cfg)
    plan, in_maps = build_host_inputs(cfg, inputs)
    nc = build_program(cfg, plan)
    if mode == "sim":
        from concourse.bass_interp import MultiCoreSim
        sim = MultiCoreSim(nc, num_cores=cfg["NCORES"])
        for cid, core in sim.cores.items():
            for k, v in in_maps[cid].items():
                core.tensor(k)[:] = v
        sim.simulate()
        return np.array(sim.cores[0].mem_tensor("out")), None
    from concourse.bass_utils import run_bass_kernel_spmd
    res = run_bass_kernel_spmd(nc, in_maps, list(range(cfg["NCORES"])),
                               trace=trace)
    return np.asarray(res.results[0]["out"]), res


def kernel(**inputs):
    out, _ = run(full_cfg(), inputs, mode="hw", trace=False)
    return out


# revision 3
# speedup vs baseline: 1.1945x; 1.0136x over previous
"""GIN graph encoder (DispatchGraphEncoder) on 8 Trainium2 NeuronCores. v3.

Gather-desc-roofline design. The dma_gather ucode costs ~8.4ns/descriptor
(byte-independent, measured), so exec time ~= total gather descriptors x
8.4ns + pipeline bubbles. v3 attacks both:

- Bucket-balanced assignment: after degree-LPT, a per-group vector-packing
  pass rebalances dst nodes across (core, window) bins to equalize
  per-(core, seg, window) edge counts across cores (padding ~9% -> ~2%).
- Replicated projection: every core computes the FULL layer-0 table
  locally from a shared permuted x^T (bf16), so layer-0 gathers start
  ~100us in with no AllGather. A small per-core own-projection fills h0.
- Pre-transposed S tiles in DRAM ([128e, p_tot, 128d]) so S loads are
  contiguous per partition (128 descs vs ~2k transposing descs each).
- DMA issue split across engines: gather-critical loads (gidx/S) on Sync,
  MLP h loads/stores + pool on DVE, weights/x/T0/collectives on Act.
  The gather stream never queues behind MLP/projection traffic.
- Cross-layer software pipelining: MLP pairs and next-layer AllGather
  chunks are emitted opportunistically inside the gather-call stream,
  across layer boundaries (no end-of-layer drain except the final one).
"""
import sys

import numpy as np
import ml_dtypes

sys.path.insert(0, "/opt/trn_rl_repo")

from concourse import bass, bacc, mybir, tile  # noqa: E402
from concourse.masks import make_identity  # noqa: E402

P = 128
AG_ENGINE = "gpsimd"  # walrus: CollectiveCompute only on DMA/Pool


def full_cfg():
    return dict(
        N=100000, E=800000, D=128, H=256, OUT=512, L=4, G=64, NCORES=8,
        RN=12500, BLK=14336, NGRP=4, CALL_TILES=8,
    )


def tiny_cfg():
    return dict(
        N=2000, E=8192, D=128, H=256, OUT=512, L=2, G=8, NCORES=8,
        RN=250, BLK=512, NGRP=2, CALL_TILES=4,
    )


def derive(cfg):
    cfg = dict(cfg)
    cfg["NW"] = cfg["BLK"] // P
    assert cfg["NW"] % cfg["NGRP"] == 0
    cfg["WG"] = cfg["NW"] // cfg["NGRP"]          # windows per group
    cfg["GROWS"] = cfg["WG"] * P                  # h rows per group per core
    cfg["SEGLEN"] = cfg["NCORES"] * cfg["GROWS"]  # table rows per segment
    assert cfg["SEGLEN"] <= 32767
    cfg["TROWS"] = cfg["NGRP"] * cfg["SEGLEN"]
    nw = cfg["NW"]
    base, extra = cfg["RN"] // nw, cfg["RN"] % nw
    cfg["SCHED"] = [base + 1 if w < extra else base for w in range(nw)]
    assert max(cfg["SCHED"]) <= P
    return cfg


# --------------------------------------------------------------------------
# host-side preprocessing (pure index/metadata manipulation)
# --------------------------------------------------------------------------

def assign_nodes(cfg, indeg):
    """Degree-balanced LPT: node -> (core, window-slot position)."""
    import heapq
    c = cfg
    nw, ncores = c["NW"], c["NCORES"]
    sched = c["SCHED"]
    heap = []
    for core in range(ncores):
        for w in range(nw):
            heap.append((0.0, core * nw + w))
    heapq.heapify(heap)
    fill = np.zeros(ncores * nw, np.int64)
    n = len(indeg)
    node2core = np.empty(n, np.int64)
    node2pos = np.empty(n, np.int64)
    order = np.argsort(-indeg, kind="stable")
    for v in order:
        while True:
            load, b = heapq.heappop(heap)
            w = b % nw
            if fill[b] < sched[w]:
                break
        node2core[v] = b // nw
        node2pos[v] = w * P + fill[b]
        fill[b] += 1
        if fill[b] < sched[w]:
            heapq.heappush(heap, (load + float(indeg[v]), b))
    return node2core, node2pos


def rebalance_buckets(cfg, src, dst, node2core, node2pos):
    """Within each src-group, reassign dst nodes to (core, window) bins to
    equalize per-(core, s, w) in-edge counts across cores (minimizes the
    cross-core max padding in the uniform gather stream). Group membership
    (node2pos // (WG*P) // ...) is preserved so src-side vectors stay fixed.
    """
    c = cfg
    nw, ncores, ngrp, wg = c["NW"], c["NCORES"], c["NGRP"], c["WG"]
    sched = np.asarray(c["SCHED"], np.int64)
    w_of = node2pos // P
    grp_of = w_of // wg
    sgrp = grp_of[src]                       # src group per edge

    # per-dst-node 4-vector of in-edge counts by src group
    vmat = np.zeros((c["N"], ngrp), np.int64)
    np.add.at(vmat, (dst, sgrp), 1)

    new_core = node2core.copy()
    new_pos = node2pos.copy()
    for g in range(ngrp):
        nodes = np.flatnonzero(grp_of == g)
        v = vmat[nodes]                       # [M, ngrp]
        order = np.argsort(-v.sum(axis=1), kind="stable")
        wlo = g * wg
        cap = np.tile(sched[wlo:wlo + wg], (ncores, 1)).astype(np.int64)
        cnt = np.zeros((ncores, ngrp, wg), np.int64)
        fill = np.zeros((ncores, wg), np.int64)
        u = np.zeros((ngrp, wg), np.int64)
        for idx in order:
            node = nodes[idx]
            vv = v[idx]                        # [ngrp]
            # delta objective for adding vv to each (core, w):
            # sum_s max(cnt[c,s,w]+vv[s], u[s,w]) - u[s,w]
            nc_ = cnt + vv[None, :, None]
            delta = np.maximum(nc_ - u[None, :, :], 0).sum(axis=1)  # [nc, wg]
            delta = np.where(fill < cap, delta, 10**9)
            # tie-break on lowest fill to keep capacities comfortable
            flat = np.argmin(delta * 10**6 + fill, axis=None)
            ci, wi = np.unravel_index(flat, delta.shape)
            cnt[ci, :, wi] += vv
            u[:, wi] = np.maximum(u[:, wi], cnt[ci, :, wi])
            new_core[node] = ci
            new_pos[node] = (wlo + wi) * P + fill[ci, wi]
            fill[ci, wi] += 1
        assert (fill == cap).all()
    return new_core, new_pos


def preprocess(cfg, edge_index):
    """Build the uniform tile/piece program + per-core gather/S data."""
    c = cfg
    src = np.asarray(edge_index[0], dtype=np.int64)
    dst = np.asarray(edge_index[1], dtype=np.int64)

    indeg = np.bincount(dst, minlength=c["N"])
    node2core, node2pos = assign_nodes(c, indeg)
    node2core, node2pos = rebalance_buckets(c, src, dst, node2core, node2pos)

    ncores, ngrp, nw, wg = c["NCORES"], c["NGRP"], c["NW"], c["WG"]
    seglen, grows = c["SEGLEN"], c["GROWS"]

    w_of = node2pos // P
    grp_of = w_of // wg
    rel_of = node2core * grows + (w_of % wg) * P + (node2pos % P)

    owner = node2core[dst]
    s_e = grp_of[src]
    rel_e = rel_of[src]
    dw_e = node2pos[dst] // P
    dc_e = node2pos[dst] % P

    counts = np.zeros((ncores, ngrp, nw), np.int64)
    np.add.at(counts, (owner, s_e, dw_e), 1)
    u = counts.max(axis=0)                        # [ngrp, nw] uniform counts

    order = np.lexsort((dw_e, s_e, owner))
    o_s, s_s, r_s, w_s, c_s = (owner[order], s_e[order], rel_e[order],
                               dw_e[order], dc_e[order])

    key = (o_s * ngrp + s_s) * nw + w_s
    bounds = np.flatnonzero(np.diff(key)) + 1
    starts = np.concatenate(([0], bounds))
    ends = np.concatenate((bounds, [len(key)]))
    range_of = {}
    for a, b in zip(starts, ends):
        range_of[(int(o_s[a]), int(s_s[a]), int(w_s[a]))] = (int(a), int(b))

    plan = [[None] * ngrp for _ in range(ngrp)]   # [block][seg]
    t_tot = 0
    p_tot = 0
    for blk in range(ngrp):
        wlo, whi = blk * wg, (blk + 1) * wg
        for s in range(ngrp):
            us = u[s, wlo:whi]
            off = np.concatenate(([0], np.cumsum(us)))
            length = int(off[-1])
            tiles = (length + P - 1) // P
            pieces = []
            for wi in range(wg):
                a, b = int(off[wi]), int(off[wi + 1])
                if a == b:
                    continue
                ta, tb = a // P, (b - 1) // P
                for t in range(ta, tb + 1):
                    pieces.append((wlo + wi, t, a >= t * P,
                                   b <= (t + 1) * P))
            calls = []
            t0 = 0
            while t0 < tiles:
                t1 = min(t0 + c["CALL_TILES"], tiles)
                calls.append((t0, t1))
                t0 = t1
            plan[blk][s] = dict(tiles=tiles, calls=calls, pieces=pieces,
                                off=off, t_base=t_tot, p_base=p_tot)
            t_tot += tiles
            p_tot += len(pieces)

    gidx = np.zeros((ncores, P, t_tot * 8), np.int16)
    s_T = np.zeros((ncores, P, p_tot, P), ml_dtypes.bfloat16)

    for blk in range(ngrp):
        wlo = blk * wg
        for s in range(ngrp):
            pl = plan[blk][s]
            off = pl["off"]
            t_base, p_base = pl["t_base"], pl["p_base"]
            for core in range(ncores):
                stream_rel = np.zeros(pl["tiles"] * P, np.int64)
                stream_col = np.full(pl["tiles"] * P, -1, np.int64)
                for wi in range(c["WG"]):
                    a = int(off[wi])
                    rng = range_of.get((core, s, wlo + wi))
                    if rng is None:
                        continue
                    ea, eb = rng
                    n = eb - ea
                    stream_rel[a:a + n] = r_s[ea:eb]
                    stream_col[a:a + n] = c_s[ea:eb]
                jj = np.arange(pl["tiles"] * P)
                tt = jj // P
                within = jj % P
                cols = (t_base + tt) * 8 + within // 16
                rows = within % 16
                for repl in range(8):
                    gidx[core, rows + 16 * repl, cols] = (
                        stream_rel.astype(np.int16))
                for i, (w, t, first, last) in enumerate(pl["pieces"]):
                    wi = w - wlo
                    a = max(int(off[wi]), t * P)
                    b = min(int(off[wi + 1]), (t + 1) * P)
                    lo = a - t * P
                    hi = b - t * P
                    sl = stream_col[t * P + lo:t * P + hi]
                    rws = np.arange(lo, hi)
                    m = sl >= 0
                    s_T[core, rws[m], p_base + i, sl[m]] = 1

    return plan, gidx, s_T, node2core, node2pos


def build_host_inputs(cfg, inputs):
    c = cfg
    x = np.asarray(inputs["x"], np.float32)
    batch = np.asarray(inputs["batch"], np.int64)
    plan, gidx, s_T, node2core, node2pos = preprocess(c, inputs["edge_index"])

    L, H, D, OUT, G = c["L"], c["H"], c["D"], c["OUT"], c["G"]
    node_w = np.asarray(inputs["node_w"], np.float32)
    node_b = np.asarray(inputs["node_b"], np.float32)
    gw1 = np.asarray(inputs["gin_w1"], np.float32)
    gb1 = np.asarray(inputs["gin_b1"], np.float32)
    gw2 = np.asarray(inputs["gin_w2"], np.float32)
    gb2 = np.asarray(inputs["gin_b2"], np.float32)
    eps = np.asarray(inputs["eps"], np.float32)
    ow1 = np.asarray(inputs["out_w1"], np.float32)
    ob1 = np.asarray(inputs["out_b1"], np.float32)
    ow2 = np.asarray(inputs["out_w2"], np.float32)
    ob2 = np.asarray(inputs["out_b2"], np.float32)

    cnt = np.bincount(batch, minlength=G).astype(np.float32)

    # permuted full x^T in table order (shared by all cores)
    w_of = node2pos // P
    grp_of = w_of // c["WG"]
    rel_of = (node2core * c["GROWS"] + (w_of % c["WG"]) * P
              + (node2pos % P))
    grow = grp_of * c["SEGLEN"] + rel_of
    xTp = np.zeros((D, c["TROWS"]), ml_dtypes.bfloat16)
    xTp[:, grow] = x.T

    common = {
        "xTp": xTp,
        "wpT": np.ascontiguousarray(node_w.T).astype(ml_dtypes.bfloat16),
        "bpT": np.ascontiguousarray(node_b.reshape(H // P, P).T),  # [P, H/P]
        "w1T": np.ascontiguousarray(
            np.transpose(gw1, (0, 2, 1))).astype(ml_dtypes.bfloat16),
        "b1T": np.ascontiguousarray(np.transpose(
            gb1.reshape(L, H // P, P), (0, 2, 1))),          # [L, P, H/P]
        "w2T": np.ascontiguousarray(
            np.transpose(gw2, (0, 2, 1))).astype(ml_dtypes.bfloat16),
        "b2T": np.ascontiguousarray(np.transpose(
            gb2.reshape(L, H // P, P), (0, 2, 1))),
        "eps_rep": np.tile(eps.reshape(1, L), (P, 1)).astype(np.float32),
        "wo1T": np.ascontiguousarray(ow1.T),                # [H, H]
        "bo1T": np.ascontiguousarray(ob1.reshape(H // P, P).T),
        "wo2T": np.ascontiguousarray(ow2.T),                # [H, OUT]
        "bo2T": np.ascontiguousarray(ob2.reshape(OUT // P, P).T),
        "cnt_rep": np.tile(cnt.reshape(1, G), (P, 1)),
    }

    in_maps = []
    for core in range(c["NCORES"]):
        mine = np.flatnonzero(node2core == core)
        pos = node2pos[mine]
        xo = np.zeros((D, c["BLK"]), ml_dtypes.bfloat16)
        xo[:, pos] = x[mine].T
        gT = np.zeros((c["BLK"], G), ml_dtypes.bfloat16)
        gT[pos, batch[mine]] = 1.0
        m = dict(common)
        m["x_own"] = xo
        m["gidx"] = gidx[core]
        m["s_T"] = s_T[core]
        m["gT"] = gT
        in_maps.append(m)
    return plan, in_maps


# --------------------------------------------------------------------------
# device program
# --------------------------------------------------------------------------

def build_program(cfg, plan):
    c = cfg
    L, H, D, OUT, G = c["L"], c["H"], c["D"], c["OUT"], c["G"]
    NW, BLK, NGRP, WG = c["NW"], c["BLK"], c["NGRP"], c["WG"]
    GROWS, SEGLEN, TROWS = c["GROWS"], c["SEGLEN"], c["TROWS"]
    NH = H // P
    NO = OUT // P
    f32 = mybir.dt.float32
    bf16 = mybir.dt.bfloat16
    t_tot = sum(pl["tiles"] for row in plan for pl in row)
    p_tot = sum(len(pl["pieces"]) for row in plan for pl in row)

    nc = bacc.Bacc("TRN2", target_bir_lowering=False, debug=False)

    xTp_d = nc.dram_tensor("xTp", [D, TROWS], bf16, kind="ExternalInput")
    x_own = nc.dram_tensor("x_own", [D, BLK], bf16, kind="ExternalInput")
    gidx_d = nc.dram_tensor("gidx", [P, t_tot * 8], mybir.dt.int16,
                            kind="ExternalInput")
    s_d = nc.dram_tensor("s_T", [P, p_tot, P], bf16, kind="ExternalInput")
    gT_d = nc.dram_tensor("gT", [BLK, G], bf16, kind="ExternalInput")
    wpT_d = nc.dram_tensor("wpT", [D, H], bf16, kind="ExternalInput")
    bpT_d = nc.dram_tensor("bpT", [P, NH], f32, kind="ExternalInput")
    w1T_d = nc.dram_tensor("w1T", [L, H, H], bf16, kind="ExternalInput")
    b1T_d = nc.dram_tensor("b1T", [L, P, NH], f32, kind="ExternalInput")
    w2T_d = nc.dram_tensor("w2T", [L, H, H], bf16, kind="ExternalInput")
    b2T_d = nc.dram_tensor("b2T", [L, P, NH], f32, kind="ExternalInput")
    eps_d = nc.dram_tensor("eps_rep", [P, L], f32, kind="ExternalInput")
    wo1T_d = nc.dram_tensor("wo1T", [H, H], f32, kind="ExternalInput")
    bo1T_d = nc.dram_tensor("bo1T", [P, NH], f32, kind="ExternalInput")
    wo2T_d = nc.dram_tensor("wo2T", [H, OUT], f32, kind="ExternalInput")
    bo2T_d = nc.dram_tensor("bo2T", [P, NO], f32, kind="ExternalInput")
    cnt_d = nc.dram_tensor("cnt_rep", [P, G], f32, kind="ExternalInput")

    out_d = nc.dram_tensor("out", [G, OUT], f32, kind="ExternalOutput")

    h_ab = [nc.dram_tensor(f"h{i}", [BLK, H], bf16) for i in range(2)]
    T_ab = [nc.dram_tensor(f"T{i}", [TROWS, H], bf16, addr_space="Shared")
            for i in range(2)]
    pp_in = nc.dram_tensor("pp_in", [P, NH * G], f32)
    pp_out = nc.dram_tensor("pp_out", [P, NH * G], f32, addr_space="Shared")

    rg = [list(range(c["NCORES"]))]

    def emit_collective(kind, op, ins, outs):
        eng = nc.scalar if AG_ENGINE == "scalar" else nc.gpsimd
        bass.BassGpSimd.collective_compute(
            eng, kind, op, replica_groups=rg, ins=ins, outs=outs)

    with tile.TileContext(nc) as tc:
        with (
            tc.tile_pool(name="const", bufs=1) as cpool,
            tc.tile_pool(name="head", bufs=1) as hpool,
            tc.tile_pool(name="agg", bufs=1) as apool,
            tc.tile_pool(name="wt", bufs=2) as wpool,
            tc.tile_pool(name="sb", bufs=4) as sb,
            tc.tile_pool(name="idx", bufs=3) as idxp,
            tc.tile_pool(name="gb", bufs=6) as gbp,
            tc.tile_pool(name="ssb", bufs=6) as ssp,
            tc.tile_pool(name="ps", bufs=4, space="PSUM") as ps,
            tc.tile_pool(name="ps_t", bufs=2, space="PSUM") as ps_t,
            tc.tile_pool(name="ps_agg", bufs=2, space="PSUM") as ps_agg,
        ):
            ident = cpool.tile([P, P], f32)
            make_identity(nc, ident[:])
            identb = cpool.tile([P, P], bf16)
            nc.vector.tensor_copy(identb[:], ident[:])
            eps_t = cpool.tile([P, L], f32)
            nc.scalar.dma_start(out=eps_t[:], in_=eps_d[:])
            eps1p = cpool.tile([P, L], f32)
            nc.scalar.add(eps1p[:], eps_t[:], 1.0)

            # head weights preloaded up front (consumed only at the tail)
            wo1sb = []
            wo2sb = []
            for kk in range(NH):
                t1w = hpool.tile([P, H], f32, name=f"wo1_{kk}")
                nc.scalar.dma_start(out=t1w[:],
                                    in_=wo1T_d[kk * P:(kk + 1) * P, :])
                wo1sb.append(t1w)
                t2w = hpool.tile([P, OUT], f32, name=f"wo2_{kk}")
                nc.scalar.dma_start(out=t2w[:],
                                    in_=wo2T_d[kk * P:(kk + 1) * P, :])
                wo2sb.append(t2w)
            bo1sb = hpool.tile([P, NH], f32, name="bo1")
            nc.scalar.dma_start(out=bo1sb[:], in_=bo1T_d[:])
            bo2sb = hpool.tile([P, NO], f32, name="bo2")
            nc.scalar.dma_start(out=bo2sb[:], in_=bo2T_d[:])
            cntsb = cpool.tile([P, G], f32)
            nc.scalar.dma_start(out=cntsb[:], in_=cnt_d[:])

            # zero both h buffers once (pad slots inside every window)
            ZC = min(8, NW)
            zt = cpool.tile([P, ZC * H], bf16)
            nc.vector.memset(zt[:], 0)
            assert BLK % (ZC * P) == 0
            for hb in h_ab:
                for zb in range(BLK // (ZC * P)):
                    nc.sync.dma_start(
                        out=hb[zb * ZC * P:(zb + 1) * ZC * P, :].rearrange(
                            "(a p) c -> p a c", p=P),
                        in_=zt[:].rearrange("p (a c) -> p a c", c=H))

            # ------------- projection (replicated table + own h0) -------------
            wp_sb = cpool.tile([D, H], bf16)
            nc.scalar.dma_start(out=wp_sb[:], in_=wpT_d[:])
            bp_sb = cpool.tile([P, NH], f32)
            nc.scalar.dma_start(out=bp_sb[:], in_=bpT_d[:])

            CW = min(4, WG)
            assert WG % CW == 0

            def proj_chunk(src_d, col0, cw, dst_d, drow0):
                """project cw windows of src_d cols [col0, col0+cw*P) and
                store node-major into dst_d rows [drow0, drow0+cw*P)."""
                xch = sb.tile([P, CW * P], bf16, tag="xch")
                nc.sync.dma_start(out=xch[:, :cw * P],
                                  in_=src_d[:, col0:col0 + cw * P])
                h0 = []
                for mh in range(NH):
                    hps = ps.tile([P, CW * P], f32, space="PSUM", tag="mlp",
                                  name=f"hps{mh}")
                    nc.tensor.matmul(out=hps[:, :cw * P],
                                     lhsT=wp_sb[:, mh * P:(mh + 1) * P],
                                     rhs=xch[:, :cw * P], start=True,
                                     stop=True)
                    h0t = sb.tile([P, CW * P], bf16, tag=f"ph{mh}")
                    if mh == 0:
                        nc.scalar.activation(
                            h0t[:, :cw * P], hps[:, :cw * P],
                            mybir.ActivationFunctionType.Relu,
                            bias=bp_sb[:, mh:mh + 1], scale=1.0)
                    else:
                        nc.vector.tensor_scalar(
                            out=h0t[:, :cw * P], in0=hps[:, :cw * P],
                            scalar1=bp_sb[:, mh:mh + 1], scalar2=0.0,
                            op0=mybir.AluOpType.add, op1=mybir.AluOpType.max)
                    h0.append(h0t)
                hnm4 = sb.tile([P, CW * H], bf16, tag="hnm4")
                htps = ps_t.tile([P, CW * H], bf16, space="PSUM", tag="aggT")
                for wl in range(cw):
                    for mh in range(NH):
                        nc.tensor.matmul(
                            out=htps[:, wl * H + mh * P:wl * H + (mh + 1) * P],
                            lhsT=h0[mh][:, wl * P:(wl + 1) * P], rhs=identb[:],
                            is_transpose=True, start=True, stop=True)
                nc.vector.tensor_copy(hnm4[:, :cw * H], htps[:, :cw * H])
                nc.scalar.dma_start(
                    out=dst_d[drow0:drow0 + cw * P, :].rearrange(
                        "(a p) c -> p a c", p=P),
                    in_=hnm4[:, :cw * H].rearrange("p (a c) -> p a c", c=H))

            # table proj emission is interleaved with the gather stream via
            # a cursor the pipeline advances ahead of each segment's use.
            proj_state = dict(next_chunk=0)
            tw_total = TROWS // P
            chunks_per_seg = SEGLEN // (CW * P)

            def pump_proj(until_chunk):
                while (proj_state["next_chunk"] < until_chunk
                       and proj_state["next_chunk"] * CW < tw_total):
                    ci = proj_state["next_chunk"]
                    proj_chunk(xTp_d, ci * CW * P, CW, T_ab[0], ci * CW * P)
                    proj_state["next_chunk"] += 1

            # segment 0 of the table first, then own h0, then the pipeline
            # pulls the rest ahead of the gather stream.
            pump_proj(chunks_per_seg)
            for wc in range(0, NW, CW):
                proj_chunk(x_own, wc * P, CW, h_ab[0], wc * P)

            # ---------------- pipelined GIN layers ----------------
            sched = c["SCHED"]
            agg = apool.tile([P, NW * H], bf16)

            pacc = cpool.tile([P, NH * G], f32)
            nc.vector.memset(pacc[:], 0)

            # per-(blk, s) first/last segment per window (same for all layers)
            first_seg = {}
            last_seg = {}
            for blk in range(NGRP):
                for s in range(NGRP):
                    for (w, t, first, last) in plan[blk][s]["pieces"]:
                        first_seg.setdefault(w, s)
                        last_seg[w] = s

            ctx = [None] * L
            ready = []            # FIFO of (l, w0) MLP pairs
            ag_pending = []       # (l, g) AGs whose MLP pairs are emitted
            emitted_ags = set()   # (l, g) AllGather chunks emitted

            def load_layer(l):
                w1sb = []
                w2sb = []
                for kk in range(NH):
                    t1w = wpool.tile([P, H], bf16, tag=f"w1_{kk}")
                    nc.scalar.dma_start(out=t1w[:],
                                        in_=w1T_d[l, kk * P:(kk + 1) * P, :])
                    w1sb.append(t1w)
                    t2w = wpool.tile([P, H], bf16, tag=f"w2_{kk}")
                    nc.scalar.dma_start(out=t2w[:],
                                        in_=w2T_d[l, kk * P:(kk + 1) * P, :])
                    w2sb.append(t2w)
                b1sb = wpool.tile([P, NH], f32, tag="b1")
                nc.scalar.dma_start(out=b1sb[:], in_=b1T_d[l])
                b2sb = wpool.tile([P, NH], f32, tag="b2")
                nc.scalar.dma_start(out=b2sb[:], in_=b2T_d[l])
                ieps = wpool.tile([P, P], bf16, tag="ieps")
                nc.scalar.activation(ieps[:], identb[:],
                                     mybir.ActivationFunctionType.Copy,
                                     bias=0.0, scale=eps1p[:, l:l + 1])
                done_w = [w not in last_seg for w in range(NW)]
                ctx[l] = dict(w1=w1sb, w2=w2sb, b1=b1sb, b2=b2sb, ieps=ieps,
                              done=done_w, next_w=0, emit_w=0)

            def emit_ag(l_next, g):
                hb = h_ab[l_next % 2]
                Tb = T_ab[l_next % 2]
                emit_collective(
                    "AllGather", mybir.AluOpType.bypass,
                    ins=[hb[g * GROWS:(g + 1) * GROWS, :]],
                    outs=[Tb[g * SEGLEN:(g + 1) * SEGLEN, :]])
                emitted_ags.add((l_next, g))

            def write_pair(l, w0, hfm_cols):
                """hfm_cols(mh, wl) -> [P, P] AP feature-major; transpose
                both windows -> node-major, one store, optional pooling."""
                hb = h_ab[(l + 1) % 2]
                hnm2 = sb.tile([P, 2 * H], bf16, tag="hnm2")
                for wl in range(2):
                    htps = ps_t.tile([P, H], bf16, space="PSUM", tag="aggT")
                    for mh in range(NH):
                        nc.tensor.matmul(
                            out=htps[:, mh * P:(mh + 1) * P],
                            lhsT=hfm_cols(mh, wl), rhs=identb[:],
                            is_transpose=True, start=True, stop=True)
                    nc.vector.tensor_copy(hnm2[:, wl * H:(wl + 1) * H],
                                          htps[:])
                nc.sync.dma_start(
                    out=hb[w0 * P:(w0 + 2) * P, :].rearrange(
                        "(a p) c -> p a c", p=P),
                    in_=hnm2[:].rearrange("p (a c) -> p a c", c=H))
                if l == L - 1:
                    for wl in range(2):
                        w = w0 + wl
                        gtw = sb.tile([P, G], bf16, tag="gtw")
                        nc.sync.dma_start(out=gtw[:],
                                          in_=gT_d[w * P:(w + 1) * P, :])
                        ppw = ps.tile([P, NH * G], f32, space="PSUM",
                                      tag="mlp")
                        for mh in range(NH):
                            nc.tensor.matmul(
                                out=ppw[:, mh * G:(mh + 1) * G],
                                lhsT=hnm2[:, wl * H + mh * P:
                                          wl * H + (mh + 1) * P],
                                rhs=gtw[:], start=True, stop=True)
                        nc.vector.tensor_add(pacc[:], pacc[:], ppw[:])

            def mlp_pair(l, w0):
                st = ctx[l]
                hb = h_ab[l % 2]
                zTp = sb.tile([P, 2 * H], bf16, tag="zTp")
                for wl in range(2):
                    w = w0 + wl
                    hw = sb.tile([P, H], bf16, tag="hw")
                    nc.sync.dma_start(out=hw[:],
                                        in_=hb[w * P:(w + 1) * P, :])
                    zps = ps.tile([P, H], f32, space="PSUM", tag="mlp")
                    aggT = ps_t.tile([P, H], bf16, space="PSUM", tag="aggT")
                    for kk in range(NH):
                        nc.tensor.matmul(
                            out=aggT[:, kk * P:(kk + 1) * P],
                            lhsT=agg[:, w * H + kk * P:w * H + (kk + 1) * P],
                            rhs=identb[:], is_transpose=True,
                            start=True, stop=True)
                        nc.tensor.matmul(
                            out=zps[:, kk * P:(kk + 1) * P],
                            lhsT=hw[:, kk * P:(kk + 1) * P], rhs=st["ieps"][:],
                            start=True, stop=True)
                    aggTs = sb.tile([P, H], bf16, tag="aggTs")
                    nc.vector.tensor_copy(aggTs[:], aggT[:])
                    zview = zTp[:].rearrange(
                        "p (kk two pp) -> p kk two pp", two=2, pp=P)
                    nc.vector.tensor_add(
                        zview[:, :, wl, :],
                        zps[:].rearrange("p (kk pp) -> p kk pp", pp=P),
                        aggTs[:].rearrange("p (kk pp) -> p kk pp", pp=P))
                H2 = 2 * H
                y1ps = ps.tile([P, H2], f32, space="PSUM", tag="mlp")
                for mh in range(NH):
                    for kk in range(NH):
                        nc.tensor.matmul(
                            out=y1ps[:, mh * 2 * P:(mh + 1) * 2 * P],
                            lhsT=st["w1"][kk][:, mh * P:(mh + 1) * P],
                            rhs=zTp[:, kk * 2 * P:(kk + 1) * 2 * P],
                            start=(kk == 0), stop=(kk == NH - 1))
                y1 = sb.tile([P, H2], bf16, tag="y1")
                for mh in range(NH):
                    nc.scalar.activation(
                        y1[:, mh * 2 * P:(mh + 1) * 2 * P],
                        y1ps[:, mh * 2 * P:(mh + 1) * 2 * P],
                        mybir.ActivationFunctionType.Relu,
                        bias=st["b1"][:, mh:mh + 1], scale=1.0)
                y2ps = ps.tile([P, H2], f32, space="PSUM", tag="mlp")
                for mh in range(NH):
                    for kk in range(NH):
                        nc.tensor.matmul(
                            out=y2ps[:, mh * 2 * P:(mh + 1) * 2 * P],
                            lhsT=st["w2"][kk][:, mh * P:(mh + 1) * P],
                            rhs=y1[:, kk * 2 * P:(kk + 1) * 2 * P],
                            start=(kk == 0), stop=(kk == NH - 1))
                h2 = sb.tile([P, H2], bf16, tag="h2")
                for mh in range(NH):
                    nc.scalar.activation(
                        h2[:, mh * 2 * P:(mh + 1) * 2 * P],
                        y2ps[:, mh * 2 * P:(mh + 1) * 2 * P],
                        mybir.ActivationFunctionType.Relu,
                        bias=st["b2"][:, mh:mh + 1], scale=1.0)
                write_pair(l, w0, lambda mh, wl: h2[:, mh * 2 * P + wl * P:
                                                    mh * 2 * P + (wl + 1) * P])

            def note_done(l, w):
                st = ctx[l]
                st["done"][w] = True
                while (st["next_w"] < NW and st["done"][st["next_w"]]
                       and st["done"][st["next_w"] + 1]):
                    ready.append((l, st["next_w"]))
                    st["next_w"] += 2

            def pump(n):
                while n > 0 and ready:
                    l, w0 = ready.pop(0)
                    mlp_pair(l, w0)
                    st = ctx[l]
                    st["emit_w"] = w0 + 2
                    if st["emit_w"] % WG == 0 and l + 1 < L:
                        ag_pending.append((l + 1, st["emit_w"] // WG - 1))
                    n -= 1

            def flush_ags(step=None):
                for ent in list(ag_pending):
                    al, g = ent
                    cons = al * NGRP * NGRP + g   # step of (al, blk0, s=g)
                    if step is None or cons <= step + 1:
                        ag_pending.remove(ent)
                        emit_ag(al, g)

            def ensure_ag(l, s):
                if l == 0:
                    return
                while (l, s) not in emitted_ags and (l, s) not in ag_pending:
                    assert ready, f"pipeline deadlock at layer {l} seg {s}"
                    pump(1)
                if (l, s) not in emitted_ags:
                    flush_ags()

            for l in range(L):
                load_layer(l)
                Tb = T_ab[l % 2]
                for w in range(NW):
                    if w not in first_seg:
                        zagg = sb.tile([P, H], bf16, tag="zagg")
                        nc.vector.memset(zagg[:], 0)
                        nc.vector.tensor_copy(agg[:, w * H:(w + 1) * H],
                                              zagg[:])
                for blk in range(NGRP):
                    for s in range(NGRP):
                        flush_ags((l * NGRP + blk) * NGRP + s)
                        ensure_ag(l, s)
                        # all of segment s's table writes must be emitted
                        # before its layer-0 gathers (Tile deps need the
                        # writes to exist first)
                        if l == 0:
                            pump_proj((s + 1) * chunks_per_seg)
                        pl = plan[blk][s]
                        idxt = idxp.tile([P, max(pl["tiles"], 1) * 8],
                                         mybir.dt.int16, tag="idxt")
                        if pl["tiles"]:
                            nc.sync.dma_start(
                                out=idxt[:, :pl["tiles"] * 8],
                                in_=gidx_d[:, pl["t_base"] * 8:
                                           (pl["t_base"] + pl["tiles"]) * 8])
                        npieces = len(pl["pieces"])
                        pieces = pl["pieces"]
                        pi = 0
                        run_ps = None
                        for (t0, t1) in pl["calls"]:
                            ct = t1 - t0
                            gb = gbp.tile([P, ct * H], bf16, tag="gbuf")
                            nc.gpsimd.dma_gather(
                                out_ap=gb[:].rearrange(
                                    "p (t d) -> p t d", d=H),
                                in_ap=Tb[s * SEGLEN:(s + 1) * SEGLEN, :],
                                idxs_ap=idxt[:, t0 * 8:t1 * 8],
                                num_idxs=ct * P, num_idxs_reg=ct * P,
                                elem_size=H)
                            pi0 = pi
                            pi1 = pi
                            while pi1 < npieces and pieces[pi1][1] < t1:
                                pi1 += 1
                            cp_n = pi1 - pi0
                            ssb = ssp.tile([P, max(cp_n, 1) * P], bf16,
                                           tag="stile")
                            if cp_n:
                                g0 = pl["p_base"] + pi0
                                nc.sync.dma_start(
                                    out=ssb[:, :cp_n * P].rearrange(
                                        "e (t d) -> e t d", d=P),
                                    in_=s_d[:, g0:g0 + cp_n, :])
                            while pi < pi1:
                                (w, t, first, last) = pieces[pi]
                                if first:
                                    run_ps = ps_agg.tile(
                                        [P, H], f32, space="PSUM",
                                        tag="aggps")
                                nc.tensor.matmul(
                                    out=run_ps[:],
                                    lhsT=ssb[:, (pi - pi0) * P:
                                             (pi - pi0 + 1) * P],
                                    rhs=gb[:, (t - t0) * H:(t - t0 + 1) * H],
                                    start=first, stop=last)
                                if last:
                                    wsl = agg[:, w * H:(w + 1) * H]
                                    if s == first_seg[w]:
                                        nc.vector.tensor_copy(wsl, run_ps[:])
                                    else:
                                        nc.vector.tensor_add(wsl, wsl,
                                                             run_ps[:])
                                    if s == last_seg[w]:
                                        note_done(l, w)
                                pi += 1
                            pump(2)
                        assert pi == npieces
            # final drain
            pump(10**9)
            flush_ags()
            assert not ready

            # ---------------- pooling + head ----------------
            nc.sync.dma_start(out=pp_in[:], in_=pacc[:])
            emit_collective("AllReduce", mybir.AluOpType.add,
                            ins=[pp_in[:]], outs=[pp_out[:]])
            ppsb = sb.tile([P, NH * G], f32, tag="ppsb")
            nc.sync.dma_start(out=ppsb[:], in_=pp_out[:])

            cnt2 = cpool.tile([P, G], f32)
            nc.vector.tensor_scalar(out=cnt2[:], in0=cntsb[:], scalar1=1.0,
                                    scalar2=None, op0=mybir.AluOpType.max)
            rec = cpool.tile([P, G], f32)
            nc.vector.reciprocal(rec[:], cnt2[:])
            hg = sb.tile([P, NH * G], f32, tag="hg")
            for mh in range(NH):
                nc.vector.tensor_mul(hg[:, mh * G:(mh + 1) * G],
                                     ppsb[:, mh * G:(mh + 1) * G], rec[:])

            o1ps = ps.tile([P, NH * G], f32, space="PSUM", tag="mlp")
            for mh in range(NH):
                for kk in range(NH):
                    nc.tensor.matmul(
                        out=o1ps[:, mh * G:(mh + 1) * G],
                        lhsT=wo1sb[kk][:, mh * P:(mh + 1) * P],
                        rhs=hg[:, kk * G:(kk + 1) * G],
                        start=(kk == 0), stop=(kk == NH - 1))
            o1 = sb.tile([P, NH * G], f32, tag="o1")
            for mh in range(NH):
                nc.scalar.activation(
                    o1[:, mh * G:(mh + 1) * G], o1ps[:, mh * G:(mh + 1) * G],
                    mybir.ActivationFunctionType.Relu,
                    bias=bo1sb[:, mh:mh + 1], scale=1.0)
            o2ps = ps.tile([P, NO * G], f32, space="PSUM", tag="mlp")
            for mq in range(NO):
                for kk in range(NH):
                    nc.tensor.matmul(
                        out=o2ps[:, mq * G:(mq + 1) * G],
                        lhsT=wo2sb[kk][:, mq * P:(mq + 1) * P],
                        rhs=o1[:, kk * G:(kk + 1) * G],
                        start=(kk == 0), stop=(kk == NH - 1))
            o2 = sb.tile([P, NO * G], f32, tag="o2")
            for mq in range(NO):
                nc.vector.tensor_scalar_add(
                    o2[:, mq * G:(mq + 1) * G], o2ps[:, mq * G:(mq + 1) * G],
                    bo2sb[:, mq:mq + 1])
            otps = ps.tile([G, OUT], f32, space="PSUM", tag="mlp")
            for mq in range(NO):
                nc.tensor.matmul(
                    out=otps[:, mq * P:(mq + 1) * P],
                    lhsT=o2[:, mq * G:(mq + 1) * G], rhs=ident[:],
                    is_transpose=True, start=True, stop=True)
            osb = sb.tile([G, OUT], f32, tag="osb")
            nc.vector.tensor_copy(osb[:], otps[:])
            nc.sync.dma_start(out=out_d[:], in_=osb[:])

    nc.compile()
    return nc


# --------------------------------------------------------------------------
# public entry
# --------------------------------------------------------------------------

def run(cfg, inputs, mode="hw", trace=False):
    cfg = derive(cfg)
    plan, in_maps = build_host_inputs(cfg, inputs)
    nc = build_program(cfg, plan)
    if mode == "sim":
        from concourse.bass_interp import MultiCoreSim
        sim = MultiCoreSim(nc, num_cores=cfg["NCORES"])
        for cid, core in sim.cores.items():
            for k, v in in_maps[cid].items():
                core.tensor(k)[:] = v
        sim.simulate()
        return np.array(sim.cores[0].mem_tensor("out")), None
    from concourse.bass_utils import run_bass_kernel_spmd
    res = run_bass_kernel_spmd(nc, in_maps, list(range(cfg["NCORES"])),
                               trace=trace)
    return np.asarray(res.results[0]["out"]), res


def kernel(**inputs):
    out, _ = run(full_cfg(), inputs, mode="hw", trace=False)
    return out
